# revision 1
# baseline (speedup 1.0000x reference)
"""Trainium2 Bass kernel for the DNC memory-step problem (nn_DNC_3882650436261).

Strategy: pure data-parallel over batch (128 -> 16 items x 8 cores).
Per core, one Tile program computes the whole DNC step using an algebraic
decomposition (validated in decomp.py) that avoids materializing the updated
link matrix and updated memory:

  - sort-free allocation: alloc_i = (1-u_i) * exp(sum_{u_j<u_i} log u_j),
    via a strict rank mask (inputs are tie-free) built on DVE and summed on PE.
  - link products: fwd/bwd read weightings expanded so the link matrix is
    consumed only by matmuls (L natural + on-chip PE transposes for L^T).
  - mem_new never materialized: cosine numerators, new norms and read vectors
    expanded into products against the OLD memory.

Hardware layout rules honored here:
  - compute-engine ops are lane-pure (same partition window for all operands);
    partition remapping happens only via PE transpose, gpsimd partition
    broadcast, or SBUF->SBUF DMA.
  - matmul outputs land at PSUM partition base 0; per-item results are copied
    PSUM->SBUF lane-pure, then DMA'd into batched (16, r*512) collectors.

Self-contained: hardcodes all shapes; no file reads.
"""
import os
import numpy as np
from contextlib import ExitStack

import concourse.bass as bass
import concourse.tile as tile
from concourse import bacc, mybir
from concourse.bass_utils import run_bass_kernel_spmd

F32 = mybir.dt.float32
AF = mybir.ActivationFunctionType
OP = mybir.AluOpType
AX = mybir.AxisListType

B, M, W, R, IN = 128, 512, 128, 4, 1024
NCORES = 8
BL = B // NCORES            # 16 items per core
DELTA = 1e-6
NCH = M // 128              # 4 chunks of the slot dim
KIN = 1152                  # padded contraction dim (1024 + bias row -> 9*128)
DTOT = 919

_dims = dict(rk=R * W, rs=R, wk=W, ws=1, ev=W, wv=W, fg=R, ag=1, wg=1, rm=3 * R)
_ofs = {}
_o = 0
for _n, _d in _dims.items():
    _ofs[_n] = _o
    _o += _d
assert _o == DTOT


def _emit(nc, tc, ctx, d):
    STOP = int(os.environ.get("KSTOP", "8"))

    def _bail(lvl):
        if STOP <= lvl:
            nc.sync.dma_start(d["out"][:], gates[:, 0:R * W])
            return True
        return False
    P = ctx.enter_context(tc.tile_pool(name="persist", bufs=1))
    ps = ctx.enter_context(tc.tile_pool(name="ps", bufs=7, space=bass.MemorySpace.PSUM))
    stg = ctx.enter_context(tc.tile_pool(name="stg", bufs=2))
    memp = ctx.enter_context(tc.tile_pool(name="memp", bufs=2))
    mts = ctx.enter_context(tc.tile_pool(name="mts", bufs=2))
    lp = ctx.enter_context(tc.tile_pool(name="lp", bufs=2))
    lts = ctx.enter_context(tc.tile_pool(name="lts", bufs=2))
    mlp = ctx.enter_context(tc.tile_pool(name="mlp", bufs=2))
    urp = ctx.enter_context(tc.tile_pool(name="urp", bufs=1))
    b64 = ctx.enter_context(tc.tile_pool(name="b64", bufs=4))
    b16 = ctx.enter_context(tc.tile_pool(name="b16", bufs=6))

    def pst(pr, fr):
        return ps.tile([pr, fr], F32, tag="ps", name="pst")

    def t64(_tag=None):
        return b64.tile([BL, R * M], F32, tag="b64", name="t64")

    def t16(_tag=None):
        return b16.tile([BL, M], F32, tag="b16", name="t16")

    def anyc(out, in_):
        nc.any.tensor_copy(out, in_)

    def bc4(t, n=M):
        # (16, R) per-(item,r) scalars -> (16, R, n) free-broadcast view
        return t.rearrange("i (r o) -> i r o", o=1).broadcast_to([BL, R, n])

    def bcm(t, n=M):
        # (16, n) per-item map -> (16, R, n) broadcast over r
        return t.rearrange("i (o m) -> i o m", o=1).broadcast_to([BL, R, n])

    def g4(t, n=M):
        # (16, R*n) flat -> (16, R, n)
        return t.rearrange("i (r m) -> i r m", m=n)

    # ---------------- constants ----------------
    consts = P.tile([128, 129], F32, tag="consts")
    nc.sync.dma_start(consts[:], d["consts"][:])
    I128 = consts[:, 0:128]
    ONES = consts[:, 128:129]

    def ptrans(out_psum, in_sb):
        p = in_sb.shape[0]
        nc.tensor.transpose(out_psum, in_sb, I128[0:p, 0:p])

    # ---------------- phase A: fused linear + gates ----------------
    xta = P.tile([128, 9, BL], F32, tag="xta")
    nc.sync.dma_start(xta[:], d["xta"][:].rearrange("(k p) i -> p k i", p=128))

    zps = pst(BL, 512)
    zps2 = pst(BL, DTOT - 512)
    with tc.tile_pool(name="wstream", bufs=2) as wp:
        for k in range(9):
            wk_t = wp.tile([128, DTOT], F32, tag="w")
            nc.sync.dma_start(wk_t[:], d["wta"][128 * k:128 * (k + 1), :])
            nc.tensor.matmul(zps[:], xta[:, k, :], wk_t[:, 0:512],
                             start=(k == 0), stop=(k == 8))
            nc.tensor.matmul(zps2[:], xta[:, k, :], wk_t[:, 512:DTOT],
                             start=(k == 0), stop=(k == 8))

    gates = P.tile([BL, DTOT], F32, tag="gates")

    def zsl(a, b):
        if b <= 512:
            return zps[:, a:b]
        assert a >= 512
        return zps2[:, a - 512:b - 512]

    def gsl(name, a=0, b=None):
        o = _ofs[name]
        if b is None:
            b = _dims[name]
        return gates[:, o + a:o + b]

    nc.scalar.activation(gates[:, 0:512], zsl(0, 512), AF.Tanh)                  # rk
    nc.scalar.activation(gates[:, 516:644], zsl(516, 644), AF.Tanh)              # wk
    nc.scalar.activation(gates[:, 645:773], zsl(645, 773), AF.Sigmoid)           # ev
    nc.scalar.activation(gates[:, 773:901], zsl(773, 901), AF.Tanh)              # wv
    nc.scalar.activation(gates[:, 901:907], zsl(901, 907), AF.Sigmoid)           # fg,ag,wg
    nc.scalar.activation(gates[:, 907:919], zsl(907, 919), AF.Identity)          # rm logits
    # softplus(z) = relu(z) + ln(1 + exp(-|z|))   (no softplus in the act tables)
    for (a, b) in [(512, 516), (644, 645)]:
        spt = b16.tile([BL, b - a], F32, tag="scrw", name="spt", bufs=2)
        nc.scalar.activation(spt[:], zsl(a, b), AF.Abs)
        nc.scalar.activation(spt[:], spt[:], AF.Exp, scale=-1.0)
        nc.scalar.activation(spt[:], spt[:], AF.Ln, bias=1.0)
        nc.scalar.activation(gates[:, a:b], zsl(a, b), AF.Relu)
        nc.vector.tensor_tensor(gates[:, a:b], gates[:, a:b], spt[:], op=OP.add)

    # read-mode softmax over groups of 3 (free-dim groups; lane-pure)
    rmz = gates[:, 907:919].rearrange("i (r k) -> i r k", k=3)
    negmax3 = P.tile([BL, R], F32, tag="negmax3")
    nc.vector.tensor_reduce(negmax3[:], rmz, axis=AX.X, op=OP.max, negate=True)
    rme = P.tile([BL, 3 * R], F32, tag="rme")
    nc.vector.tensor_tensor(rme[:].rearrange("i (r k) -> i r k", k=3), rmz,
                            negmax3[:].rearrange("i (r o) -> i r o", o=1).broadcast_to([BL, R, 3]),
                            op=OP.add)
    nc.scalar.activation(rme[:], rme[:], AF.Exp)
    rmsum = P.tile([BL, R], F32, tag="rmsum")
    nc.vector.tensor_reduce(rmsum[:], rme[:].rearrange("i (r k) -> i r k", k=3), axis=AX.X, op=OP.add)
    nc.vector.reciprocal(rmsum[:], rmsum[:])
    rm = P.tile([BL, 3 * R], F32, tag="rm")
    nc.vector.tensor_tensor(rm[:].rearrange("i (r k) -> i r k", k=3),
                            rme[:].rearrange("i (r k) -> i r k", k=3),
                            rmsum[:].rearrange("i (r o) -> i r o", o=1).broadcast_to([BL, R, 3]),
                            op=OP.mult)
    # per-(i,r) mode scalars (16, R) each, k-th mode
    mode = []
    for k in range(3):
        mk = P.tile([BL, R], F32, tag=f"mode{k}")
        anyc(mk[:], rm[:].rearrange("i (r k) -> i r k", k=3)[:, :, k])
        mode.append(mk)

    if _bail(1):
        return
    # ---------------- phase A2: usage / u ----------------
    rw16 = P.tile([BL, R * M], F32, tag="rw16")      # (16, 2048) read_weights
    nc.sync.dma_start(rw16[:], d["rw"][:])
    wwin = t16()
    nc.sync.dma_start(wwin[:], d["wwin"][:])
    usg = t16()
    nc.sync.dma_start(usg[:], d["usage"][:])
    prec = P.tile([BL, M], F32, tag="prec")
    nc.sync.dma_start(prec[:], d["prec"][:])
    rwt = P.tile([128, NCH, BL * R], F32, tag="rwt")  # rw^T: [n-part, chunk, 4i+r]
    nc.sync.dma_start(rwt[:], d["rwt"][:].rearrange("(c p) j -> p c j", p=128))

    # psi = prod_r (fg_r*rw_r - 1)   (4 sign flips cancel)
    psi4 = t64()
    nc.vector.tensor_tensor(g4(psi4[:]), g4(rw16[:]),
                            bc4(gsl("fg")), op=OP.mult)
    nc.any.tensor_scalar(psi4[:], psi4[:], 1.0, None, op0=OP.subtract)
    psi = t16()
    nc.vector.tensor_tensor(psi[:], psi4[:, 0:M], psi4[:, M:2 * M], op=OP.mult)
    nc.vector.tensor_tensor(psi4[:, 2 * M:3 * M], psi4[:, 2 * M:3 * M], psi4[:, 3 * M:4 * M], op=OP.mult)
    nc.vector.tensor_tensor(psi[:], psi[:], psi4[:, 2 * M:3 * M], op=OP.mult)

    u_sb = P.tile([BL, M], F32, tag="u_sb")
    nc.vector.tensor_tensor(u_sb[:], usg[:], wwin[:], op=OP.mult)
    nc.vector.tensor_tensor(u_sb[:], usg[:], u_sb[:], op=OP.subtract)
    nc.vector.tensor_tensor(u_sb[:], u_sb[:], wwin[:], op=OP.add)
    nc.vector.tensor_tensor(u_sb[:], u_sb[:], psi[:], op=OP.mult)
    nc.any.tensor_scalar(u_sb[:], u_sb[:], 1.0 - DELTA, DELTA, op0=OP.mult, op1=OP.add)

    # u^T / log(u^T): UT/LUT (128, NCH*BL) col 16c+i
    UT = P.tile([128, NCH * BL], F32, tag="UT")
    LUT = P.tile([128, NCH * BL], F32, tag="LUT")
    for c in range(NCH):
        utp = pst(128, BL)
        ptrans(utp[:], u_sb[:, 128 * c:128 * (c + 1)])
        anyc(UT[:, BL * c:BL * (c + 1)], utp[:])
    nc.scalar.activation(LUT[:], UT[:], AF.Ln)

    # gate transposes -> packed per-item lhsT tensors
    # KCM (128, 11*BL): per item cols [wk | rk0..3 | rk*ev 0..3 | wv | ev*wv]
    # NRM (128, 3*BL): per item cols [1 | ev | ev^2]
    KCM = P.tile([128, 11 * BL], F32, tag="KCM")
    NRM = P.tile([128, 3 * BL], F32, tag="NRM")
    EVT = P.tile([128, BL], F32, tag="EVT")

    def kcm_col(j):
        return KCM[:].rearrange("p (i k) -> p i k", k=11)[:, :, j]

    gtp = pst(128, BL)
    ptrans(gtp[:], gsl("wk"))
    anyc(kcm_col(0), gtp[:])
    gtp = pst(128, BL)
    ptrans(gtp[:], gsl("ev"))
    anyc(EVT[:], gtp[:])
    gtp = pst(128, BL)
    ptrans(gtp[:], gsl("wv"))
    anyc(kcm_col(9), gtp[:])
    nc.vector.tensor_tensor(kcm_col(10), kcm_col(9), EVT[:], op=OP.mult)  # ev*wv
    for r in range(R):
        gtp = pst(128, BL)
        ptrans(gtp[:], gsl("rk", r * W, (r + 1) * W))
        anyc(kcm_col(1 + r), gtp[:])
        nc.vector.tensor_tensor(kcm_col(5 + r), kcm_col(1 + r), EVT[:], op=OP.mult)
    nc.any.memset(NRM[:].rearrange("p (i k) -> p i k", k=3)[:, :, 0], 1.0)
    anyc(NRM[:].rearrange("p (i k) -> p i k", k=3)[:, :, 1], EVT[:])
    nc.scalar.activation(NRM[:].rearrange("p (i k) -> p i k", k=3)[:, :, 2], EVT[:], AF.Square)

    # per-item key-norm scalars
    scr = b16.tile([BL, W], F32, tag="scrw", name="scr", bufs=2)
    bw128 = P.tile([BL, 1], F32, tag="bw128")
    nc.scalar.activation(scr[:], gsl("wk"), AF.Square, accum_out=bw128[:])
    nc.scalar.activation(bw128[:], bw128[:], AF.Sqrt)
    nc.any.tensor_scalar(bw128[:], bw128[:], float(W), float(W) * DELTA, op0=OP.mult, op1=OP.add)
    bnr = P.tile([BL, R], F32, tag="bnr")
    rkwv = P.tile([BL, R], F32, tag="rkwv")
    for r in range(R):
        nc.scalar.activation(scr[:], gsl("rk", r * W, (r + 1) * W), AF.Square, accum_out=bnr[:, r:r + 1])
        nc.vector.tensor_tensor(scr[:], gsl("rk", r * W, (r + 1) * W), gsl("wv"), op=OP.mult)
        nc.vector.tensor_scalar(scr[:], scr[:], 1.0, None, op0=OP.mult, op1=OP.add,
                                accum_out=rkwv[:, r:r + 1])
    nc.scalar.activation(bnr[:], bnr[:], AF.Sqrt)
    nc.any.tensor_scalar(bnr[:], bnr[:], float(W), float(W) * DELTA, op0=OP.mult, op1=OP.add)
    c3 = P.tile([BL, 1], F32, tag="c3")
    nc.scalar.activation(scr[:], gsl("wv"), AF.Square, accum_out=c3[:])

    if _bail(2):
        return
    # ---------------- batched collectors (item-rows layout) ----------------
    pm16 = P.tile([BL, 3 * M], F32, tag="pm16")      # [wcwnum | T1 | T2]
    ps16 = P.tile([BL, 3 * M], F32, tag="ps16")      # [S0 | S1 | S2]
    cn16 = P.tile([BL, 2 * R * M], F32, tag="cn16")  # [cwA(4) | cwB(4)]
    e16 = P.tile([BL, M], F32, tag="e16")            # exp(s)
    bp16 = P.tile([BL, 2 * R * M], F32, tag="bp16")  # [B1(4) | B2(4)]
    fp16 = P.tile([BL, 2 * R * M], F32, tag="fp16")  # [F1(4) | F2(4)]
    rv16 = P.tile([BL, 2 * R * W], F32, tag="rv16")  # [RA(4) | RB(4)]

    # ---------------- phase B: per-item memory products + alloc mask ----------------
    for i in range(BL):
        memi = memp.tile([128, NCH, W], F32, tag="memi")
        nc.sync.dma_start(memi[:], d["mem"][i].rearrange("(c p) w -> p c w", p=128))
        mtps = pst(128, M)
        for c in range(NCH):
            ptrans(mtps[:, 128 * c:128 * (c + 1)], memi[:, c, :])
        mt = mts.tile([128, M], F32, tag="mt")
        anyc(mt[:], mtps[:])
        mt2 = mts.tile([128, M], F32, tag="mt2")
        nc.scalar.activation(mt2[:], mt[:], AF.Square)

        pmps = pst(11, M)
        nc.tensor.matmul(pmps[:], KCM[:, 11 * i:11 * (i + 1)], mt[:])
        psps = pst(3, M)
        nc.tensor.matmul(psps[:], NRM[:, 3 * i:3 * (i + 1)], mt2[:])

        # stage PSUM -> SBUF (lane-pure), then DMA into item-rows collectors
        pmst = stg.tile([11, M], F32, tag="stg")
        anyc(pmst[:], pmps[:])
        psst = stg.tile([3, M], F32, tag="stg")
        anyc(psst[:], psps[:])
        # pmst rows: [wcwnum, rk0..3, rkev0..3, T1, T2] -> pm16 [wcwnum|T1|T2], cn16 [A|B]
        nc.sync.dma_start(pm16[i:i + 1, 0:M], pmst[0:1, :])
        nc.sync.dma_start(pm16[i:i + 1, M:3 * M], pmst[9:11, :])
        nc.sync.dma_start(cn16[i:i + 1, :], pmst[1:9, :])
        nc.sync.dma_start(ps16[i:i + 1, :], psst[:])

        # alloc mask: ML[j,t] = (u_t > u_j)*logu_j ; s_t = sum_j
        urow = urp.tile([1, M], F32, tag="urow")
        nc.sync.dma_start(urow[:], u_sb[i:i + 1, :])
        urep = urp.tile([128, M], F32, tag="urep")
        nc.gpsimd.partition_broadcast(urep[:], urow[:])
        sps = pst(1, M)
        for c in range(NCH):
            ml = mlp.tile([128, M], F32, tag="ml")
            nc.vector.tensor_scalar(ml[:], urep[:], UT[:, BL * c + i:BL * c + i + 1],
                                    LUT[:, BL * c + i:BL * c + i + 1], op0=OP.is_gt, op1=OP.mult)
            nc.tensor.matmul(sps[:], ONES, ml[:], start=(c == 0), stop=(c == NCH - 1))
        est = stg.tile([1, M], F32, tag="stg")
        nc.scalar.activation(est[:], sps[:], AF.Exp)
        nc.sync.dma_start(e16[i:i + 1, :], est[:])

    if _bail(3):
        return
    # ---------------- phase C: batched content/alloc/write weights ----------------
    S0v, S1v, S2v = ps16[:, 0:M], ps16[:, M:2 * M], ps16[:, 2 * M:3 * M]
    wcwnum, T1v, T2v = pm16[:, 0:M], pm16[:, M:2 * M], pm16[:, 2 * M:3 * M]

    AO = t16()
    nc.scalar.activation(AO[:], S0v, AF.Sqrt)
    nc.any.tensor_scalar(AO[:], AO[:], 1.0, DELTA, op0=OP.mult, op1=OP.add)

    wden = t16()
    nc.any.tensor_scalar(wden[:], AO[:], bw128[:], DELTA, op0=OP.mult, op1=OP.add)
    nc.vector.reciprocal(wden[:], wden[:])
    wz = t16()
    nc.vector.tensor_tensor(wz[:], wcwnum, wden[:], op=OP.mult)
    nc.any.tensor_scalar(wz[:], wz[:], gsl("ws"), None, op0=OP.mult)
    wneg = P.tile([BL, 1], F32, tag="wneg")
    nc.vector.tensor_reduce(wneg[:], wz[:], axis=AX.X, op=OP.max, negate=True)
    wsum = P.tile([BL, 1], F32, tag="wsum")
    nc.scalar.activation(wz[:], wz[:], AF.Exp, bias=wneg[:], accum_out=wsum[:])
    nc.vector.reciprocal(wsum[:], wsum[:])
    wcw = t16()
    nc.any.tensor_scalar(wcw[:], wz[:], wsum[:], None, op0=OP.mult)

    # alloc + write weights
    onemu = t16()
    nc.any.tensor_scalar(onemu[:], u_sb[:], -1.0, 1.0, op0=OP.mult, op1=OP.add)
    alloc = t16()
    nc.vector.tensor_tensor(alloc[:], onemu[:], e16[:], op=OP.mult)
    ww = P.tile([BL, M], F32, tag="ww")
    onemag = P.tile([BL, 1], F32, tag="onemag")
    nc.any.tensor_scalar(onemag[:], gsl("ag"), -1.0, 1.0, op0=OP.mult, op1=OP.add)
    nc.any.tensor_scalar(alloc[:], alloc[:], gsl("ag"), None, op0=OP.mult)
    nc.any.tensor_scalar(ww[:], wcw[:], onemag[:], None, op0=OP.mult)
    nc.vector.tensor_tensor(ww[:], ww[:], alloc[:], op=OP.add)
    nc.any.tensor_scalar(ww[:], ww[:], gsl("wg"), None, op0=OP.mult)

    # ww^T (128, NCH*BL) col 16c+i, and 1-ww^T
    WWT = P.tile([128, NCH * BL], F32, tag="WWT")
    OWWT = P.tile([128, NCH * BL], F32, tag="OWWT")
    for c in range(NCH):
        wtp2 = pst(128, BL)
        ptrans(wtp2[:], ww[:, 128 * c:128 * (c + 1)])
        anyc(WWT[:, BL * c:BL * (c + 1)], wtp2[:])
    nc.any.tensor_scalar(OWWT[:], WWT[:], -1.0, 1.0, op0=OP.mult, op1=OP.add)

    # new norms^2 -> AN (16,512)
    ww2 = t16()
    nc.scalar.activation(ww2[:], ww[:], AF.Square)
    q1 = t16()
    nc.vector.tensor_tensor(q1[:], S1v, T1v, op=OP.subtract)
    nc.vector.tensor_tensor(q1[:], ww[:], q1[:], op=OP.mult)
    q2 = t16()
    nc.any.tensor_scalar(q2[:], T2v, -2.0, None, op0=OP.mult)
    nc.vector.tensor_tensor(q2[:], q2[:], S2v, op=OP.add)
    nc.any.tensor_scalar(q2[:], q2[:], 1.0, c3[:], op0=OP.mult, op1=OP.add)
    nc.vector.tensor_tensor(q2[:], q2[:], ww2[:], op=OP.mult)
    AN = t16()
    nc.any.tensor_scalar(q1[:], q1[:], -2.0, None, op0=OP.mult)
    nc.vector.tensor_tensor(AN[:], S0v, q1[:], op=OP.add)
    nc.vector.tensor_tensor(AN[:], AN[:], q2[:], op=OP.add)
    nc.scalar.activation(AN[:], AN[:], AF.Sqrt)
    nc.any.tensor_scalar(AN[:], AN[:], 1.0, DELTA, op0=OP.mult, op1=OP.add)

    # cw softmax in (16, R, 512) layout
    cwA = g4(cn16[:, 0:R * M])
    cwB = g4(cn16[:, R * M:2 * R * M])
    cnum = t64()
    nc.vector.tensor_tensor(g4(cnum[:]), cwB, bcm(ww[:]), op=OP.mult)
    nc.vector.tensor_tensor(cnum[:], cn16[:, 0:R * M], cnum[:], op=OP.subtract)
    ct = t64()
    nc.vector.tensor_tensor(g4(ct[:]), bcm(ww[:]), bc4(rkwv[:]), op=OP.mult)
    nc.vector.tensor_tensor(cnum[:], cnum[:], ct[:], op=OP.add)
    cden = t64()
    nc.vector.tensor_tensor(g4(cden[:]), bcm(AN[:]), bc4(bnr[:]), op=OP.mult)
    nc.any.tensor_scalar(cden[:], cden[:], 1.0, DELTA, op0=OP.mult, op1=OP.add)
    nc.vector.reciprocal(cden[:], cden[:])
    nc.vector.tensor_tensor(cnum[:], cnum[:], cden[:], op=OP.mult)
    nc.vector.tensor_tensor(g4(cnum[:]), g4(cnum[:]), bc4(gsl("rs")), op=OP.mult)
    cneg = P.tile([BL, R], F32, tag="cneg")
    nc.vector.tensor_reduce(cneg[:], g4(cnum[:]), axis=AX.X, op=OP.max, negate=True)
    nc.vector.tensor_tensor(g4(cnum[:]), g4(cnum[:]), bc4(cneg[:]), op=OP.add)
    nc.scalar.activation(cnum[:], cnum[:], AF.Exp)
    csum = P.tile([BL, R], F32, tag="csum")
    nc.vector.tensor_reduce(csum[:], g4(cnum[:]), axis=AX.X, op=OP.add)
    nc.vector.reciprocal(csum[:], csum[:])
    cw = P.tile([BL, R * M], F32, tag="cw")
    nc.vector.tensor_tensor(g4(cw[:]), g4(cnum[:]), bc4(csum[:]), op=OP.mult)

    # RWALL (128, NCH, 12*BL): per item [rw | rw*(1-ww) | rw*ww] (transposed)
    RWALL = P.tile([128, NCH, 12 * BL], F32, tag="RWALL")
    for c in range(NCH):
        rsl = rwt[:, c, :].rearrange("p (i r) -> p i r", r=R)
        dst = RWALL[:, c, :].rearrange("p (i g r) -> p i g r", g=3, r=R)
        anyc(dst[:, :, 0, :], rsl)
        wb = OWWT[:, BL * c:BL * (c + 1)].rearrange("p (i o) -> p i o", o=1).broadcast_to([128, BL, R])
        nc.vector.tensor_tensor(dst[:, :, 1, :], rsl, wb, op=OP.mult)
        wb2 = WWT[:, BL * c:BL * (c + 1)].rearrange("p (i o) -> p i o", o=1).broadcast_to([128, BL, R])
        nc.vector.tensor_tensor(dst[:, :, 2, :], rsl, wb2, op=OP.mult)

    # c_pr = p.rw_r ; d_wr = rw_r.ww  (16, R)
    cpr = P.tile([BL, R], F32, tag="cpr")
    dwr = P.tile([BL, R], F32, tag="dwr")
    scrm = t16()
    for r in range(R):
        nc.vector.tensor_tensor(scrm[:], rw16[:, r * M:(r + 1) * M], prec[:], op=OP.mult)
        nc.vector.tensor_scalar(scrm[:], scrm[:], 1.0, None, op0=OP.mult, op1=OP.add,
                                accum_out=cpr[:, r:r + 1])
        nc.vector.tensor_tensor(scrm[:], rw16[:, r * M:(r + 1) * M], ww[:], op=OP.mult)
        nc.vector.tensor_scalar(scrm[:], scrm[:], 1.0, None, op0=OP.mult, op1=OP.add,
                                accum_out=dwr[:, r:r + 1])

    if _bail(4):
        return
    # ---------------- phase D: per-item link products ----------------
    DGT = P.tile([128, NCH * BL], F32, tag="DGT")
    for i in range(BL):
        lc = lp.tile([128, NCH, M], F32, tag="lc")   # [p, a, m] = L[128a+p, m]
        nc.sync.dma_start(lc[:], d["link"][i].rearrange("(a p) m -> p a m", p=128))
        bps = pst(8, M)
        for a in range(NCH):
            lhs = RWALL[:, a, 12 * i:12 * i + 8]
            nc.tensor.matmul(bps[:], lhs, lc[:, a, :], start=(a == 0), stop=(a == NCH - 1))
        bst = stg.tile([8, M], F32, tag="stg")
        anyc(bst[:], bps[:])
        nc.sync.dma_start(bp16[i:i + 1, :], bst[:])
        # diag of chunk a lives at lc[:, a, 128a:128a+128]
        for a in range(NCH):
            dsc = mlp.tile([128, M], F32, tag="ml")
            nc.vector.tensor_tensor(dsc[:, 0:128], lc[:, a, 128 * a:128 * (a + 1)], I128, op=OP.mult)
            nc.vector.tensor_scalar(dsc[:, 0:128], dsc[:, 0:128], 1.0, None, op0=OP.mult, op1=OP.add,
                                    accum_out=DGT[:, BL * a + i:BL * a + i + 1])
        # transposes: LT chunk c collects block a at cols [128a:...]
        fps1 = pst(4, M)
        fps2 = pst(4, M)
        for c in range(NCH):
            ltps = pst(128, M)
            for a in range(NCH):
                ptrans(ltps[:, 128 * a:128 * (a + 1)], lc[:, a, 128 * c:128 * (c + 1)])
            lt = lts.tile([128, M], F32, tag="lt")
            anyc(lt[:], ltps[:])
            nc.tensor.matmul(fps1[:], RWALL[:, c, 12 * i:12 * i + 4], lt[:],
                             start=(c == 0), stop=(c == NCH - 1))
            nc.tensor.matmul(fps2[:], RWALL[:, c, 12 * i + 8:12 * i + 12], lt[:],
                             start=(c == 0), stop=(c == NCH - 1))
        fst1 = stg.tile([4, M], F32, tag="stg")
        anyc(fst1[:], fps1[:])
        nc.sync.dma_start(fp16[i:i + 1, 0:R * M], fst1[:])
        fst2 = stg.tile([4, M], F32, tag="stg")
        anyc(fst2[:], fps2[:])
        nc.sync.dma_start(fp16[i:i + 1, R * M:2 * R * M], fst2[:])

    if _bail(5):
        return
    # ---------------- phase E: assemble fwd/bwd/rw_new (all (16, R*512)) ----------------
    diag = t16()
    dgp = pst(BL, M)
    for c in range(NCH):
        ptrans(dgp[:, 128 * c:128 * (c + 1)], DGT[:, BL * c:BL * (c + 1)])
    anyc(diag[:], dgp[:])
    dv = t16()
    nc.any.tensor_scalar(dv[:], ww[:], -2.0, 1.0, op0=OP.mult, op1=OP.add)
    nc.vector.tensor_tensor(dv[:], dv[:], diag[:], op=OP.mult)
    t2 = t16()
    nc.vector.tensor_tensor(t2[:], ww[:], prec[:], op=OP.mult)
    nc.vector.tensor_tensor(dv[:], dv[:], t2[:], op=OP.add)
    DR = t64()
    nc.vector.tensor_tensor(g4(DR[:]), g4(rw16[:]), bcm(dv[:]), op=OP.mult)

    B1 = bp16[:, 0:R * M]
    B2 = bp16[:, R * M:2 * R * M]
    F1 = fp16[:, 0:R * M]
    F2 = fp16[:, R * M:2 * R * M]

    fwd = t64()
    onemw = t16()
    nc.any.tensor_scalar(onemw[:], ww[:], -1.0, 1.0, op0=OP.mult, op1=OP.add)
    nc.vector.tensor_tensor(g4(fwd[:]), g4(F1), bcm(onemw[:]), op=OP.mult)
    nc.vector.tensor_tensor(fwd[:], fwd[:], F2, op=OP.subtract)
    ft = t64()
    nc.vector.tensor_tensor(g4(ft[:]), bcm(ww[:]), bc4(cpr[:]), op=OP.mult)
    nc.vector.tensor_tensor(fwd[:], fwd[:], ft[:], op=OP.add)
    nc.vector.tensor_tensor(fwd[:], fwd[:], DR[:], op=OP.subtract)

    bwd = t64()
    nc.vector.tensor_tensor(g4(bwd[:]), g4(B1), bcm(ww[:]), op=OP.mult)
    nc.vector.tensor_tensor(bwd[:], B2, bwd[:], op=OP.subtract)
    nc.vector.tensor_tensor(g4(ft[:]), bcm(prec[:]), bc4(dwr[:]), op=OP.mult)
    nc.vector.tensor_tensor(bwd[:], bwd[:], ft[:], op=OP.add)
    nc.vector.tensor_tensor(bwd[:], bwd[:], DR[:], op=OP.subtract)

    rwnew = P.tile([BL, R * M], F32, tag="rwnew")
    nc.vector.tensor_tensor(g4(bwd[:]), g4(bwd[:]), bc4(mode[0][:]), op=OP.mult)
    nc.vector.tensor_tensor(g4(fwd[:]), g4(fwd[:]), bc4(mode[1][:]), op=OP.mult)
    nc.vector.tensor_tensor(rwnew[:], bwd[:], fwd[:], op=OP.add)
    ct2 = t64()
    nc.vector.tensor_tensor(g4(ct2[:]), g4(cw[:]), bc4(mode[2][:]), op=OP.mult)
    nc.vector.tensor_tensor(rwnew[:], rwnew[:], ct2[:], op=OP.add)

    # sc = rw_new . ww  (16, R)
    sc = P.tile([BL, R], F32, tag="sc")
    for r in range(R):
        nc.vector.tensor_tensor(scrm[:], rwnew[:, r * M:(r + 1) * M], ww[:], op=OP.mult)
        nc.vector.tensor_scalar(scrm[:], scrm[:], 1.0, None, op0=OP.mult, op1=OP.add,
                                accum_out=sc[:, r:r + 1])

    # RVL (128, NCH, 8*BL): per item [rwnT | rwnT*ww]
    RVL = P.tile([128, NCH, 8 * BL], F32, tag="RVL")
    for c in range(NCH):
        for r in range(R):
            rnp = pst(128, BL)
            ptrans(rnp[:], rwnew[:, r * M + 128 * c:r * M + 128 * (c + 1)])
            anyc(RVL[:, c, :].rearrange("p (i k) -> p i k", k=8)[:, :, r], rnp[:])
        dst = RVL[:, c, :].rearrange("p (i k) -> p i k", k=8)
        wb = WWT[:, BL * c:BL * (c + 1)].rearrange("p (i o) -> p i o", o=1).broadcast_to([128, BL, R])
        nc.vector.tensor_tensor(dst[:, :, R:2 * R], dst[:, :, 0:R], wb, op=OP.mult)

    if _bail(6):
        return
    # ---------------- phase F: read vectors ----------------
    for i in range(BL):
        memi = memp.tile([128, NCH, W], F32, tag="memi")
        nc.sync.dma_start(memi[:], d["mem"][i].rearrange("(c p) w -> p c w", p=128))
        rvp = pst(8, W)
        for c in range(NCH):
            nc.tensor.matmul(rvp[:], RVL[:, c, 8 * i:8 * (i + 1)], memi[:, c, :],
                             start=(c == 0), stop=(c == NCH - 1))
        rvst = stg.tile([8, W], F32, tag="stg")
        anyc(rvst[:], rvp[:])
        nc.sync.dma_start(rv16[i:i + 1, :], rvst[:])

    if _bail(7):
        return
    # ---------------- phase G: final rv = RA - ev*RB + (rwn.ww)*wv ----------------
    RA = g4(rv16[:, 0:R * W], n=W)
    RB = g4(rv16[:, R * W:2 * R * W], n=W)
    out_sb = P.tile([BL, R * W], F32, tag="out_sb")
    nc.vector.tensor_tensor(g4(out_sb[:], n=W), RB, bcm(gsl("ev"), n=W), op=OP.mult)
    nc.vector.tensor_tensor(g4(out_sb[:], n=W), RA, g4(out_sb[:], n=W), op=OP.subtract)
    wvt = b16.tile([BL, R * W], F32, tag="b16", name="wvt")
    nc.vector.tensor_tensor(g4(wvt[:], n=W), bcm(gsl("wv"), n=W), bc4(sc[:], n=W), op=OP.mult)
    nc.vector.tensor_tensor(out_sb[:], out_sb[:], wvt[:], op=OP.add)
    nc.sync.dma_start(d["out"][:], out_sb[:])


_NC_CACHE = {}


def build_nc():
    if "nc" in _NC_CACHE:
        return _NC_CACHE["nc"]
    nc = bacc.Bacc("TRN2", target_bir_lowering=False, debug=False)
    d = {}
    d["consts"] = nc.dram_tensor("consts", [128, 129], F32, kind="ExternalInput")
    d["xta"] = nc.dram_tensor("xta", [KIN, BL], F32, kind="ExternalInput")
    d["wta"] = nc.dram_tensor("wta", [KIN, DTOT], F32, kind="ExternalInput")
    d["mem"] = nc.dram_tensor("mem", [BL, M, W], F32, kind="ExternalInput")
    d["link"] = nc.dram_tensor("link", [BL, M, M], F32, kind="ExternalInput")
    d["prec"] = nc.dram_tensor("prec", [BL, M], F32, kind="ExternalInput")
    d["rw"] = nc.dram_tensor("rw", [BL, R * M], F32, kind="ExternalInput")
    d["rwt"] = nc.dram_tensor("rwt", [M, BL * R], F32, kind="ExternalInput")
    d["wwin"] = nc.dram_tensor("wwin", [BL, M], F32, kind="ExternalInput")
    d["usage"] = nc.dram_tensor("usage", [BL, M], F32, kind="ExternalInput")
    d["out"] = nc.dram_tensor("out", [BL, R * W], F32, kind="ExternalOutput")
    with tile.TileContext(nc) as tc:
        with ExitStack() as ctx:
            _emit(nc, tc, ctx, d)
    nc.compile()
    _NC_CACHE["nc"] = nc
    return nc


def make_in_maps(inputs):
    names = ["rk", "rs", "wk", "ws", "ev", "wv", "fg", "ag", "wg", "rm"]
    Wall = np.concatenate([np.asarray(inputs[f"W_{n}"]) for n in names], axis=0).astype(np.float32)
    ball = np.concatenate([np.asarray(inputs[f"b_{n}"]) for n in names], axis=0).astype(np.float32)
    wta = np.zeros((KIN, DTOT), np.float32)
    wta[:IN] = Wall.T
    wta[IN] = ball
    consts = np.zeros((128, 129), np.float32)
    consts[:, :128] = np.eye(128, dtype=np.float32)
    consts[:, 128] = 1.0

    x = np.asarray(inputs["x"], np.float32)
    mem = np.asarray(inputs["memory"], np.float32)
    link = np.asarray(inputs["link_matrix"], np.float32)[:, 0]
    prec = np.asarray(inputs["precedence"], np.float32)[:, 0]
    rw = np.asarray(inputs["read_weights"], np.float32)
    wwin = np.asarray(inputs["write_weights"], np.float32)[:, 0]
    usage = np.asarray(inputs["usage_vector"], np.float32)

    in_maps = []
    for c in range(NCORES):
        sl = slice(c * BL, (c + 1) * BL)
        xta = np.zeros((KIN, BL), np.float32)
        xta[:IN] = x[sl].T
        xta[IN] = 1.0
        rws = rw[sl]
        in_maps.append({
            "consts": consts,
            "xta": xta,
            "wta": wta,
            "mem": np.ascontiguousarray(mem[sl]),
            "link": np.ascontiguousarray(link[sl]),
            "prec": np.ascontiguousarray(prec[sl]),
            "rw": np.ascontiguousarray(rws.reshape(BL, R * M)),
            "rwt": np.ascontiguousarray(rws.transpose(2, 0, 1).reshape(M, BL * R)),
            "wwin": np.ascontiguousarray(wwin[sl]),
            "usage": np.ascontiguousarray(usage[sl]),
        })
    return in_maps


def kernel(**inputs):
    nc = build_nc()
    in_maps = make_in_maps(inputs)
    res = run_bass_kernel_spmd(nc, in_maps, list(range(NCORES))).results
    out = np.concatenate([res[c]["out"].reshape(BL, R, W) for c in range(NCORES)], axis=0)
    return out.astype(np.float32)



# revision 2
# speedup vs baseline: 1.0388x; 1.0388x over previous
"""Trainium2 Bass kernel v2 for the DNC memory-step problem (nn_DNC_3882650436261).

Pure data-parallel over batch (128 -> 16 items x 8 cores). Same algebraic
decomposition as v1 (sort-free allocation, link products via matmuls against
L / L^T, mem_new never materialized), but with a new hardware mapping:

  - "m-part world": the 512-slot dim lives on partitions (4 chunks of 128);
    all elementwise/softmax math runs at full 128-lane width with free dim
    = (chunk, item[, read-head]).  Softmax sums over slots use a free-dim
    reduce over chunks + gpsimd partition_all_reduce.
  - big matmuls in bf16 (1 cycle/row on PE instead of 4 for fp32); link
    matmuls in fp8e4m3 with inputs pre-scaled by 512 (host) / 512 (rw,
    on-chip) to stay in fp8 normal range; PSUM results rescaled by 2^-18.
  - L and L^T both streamed from HBM (host-prepped, fp8) -> zero on-chip
    128x128 PE transposes of the link matrix.
  - per-item [16,512] item-part matmul results are scattered into m-part
    collectors with single dma_start_transpose (hw XBAR) instructions.
  - the sort-free allocation mask runs as one fused scalar_tensor_tensor
    (compare * log-u, accumulate over free dim) per (item, chunk).

Self-contained: hardcodes all shapes; no file reads.
"""
import os
import numpy as np
import ml_dtypes
from contextlib import ExitStack

import concourse.bass as bass
import concourse.tile as tile
import concourse.bass_isa as bass_isa
from concourse import bacc, mybir
from concourse.bass_utils import run_bass_kernel_spmd

F32 = mybir.dt.float32
BF16 = mybir.dt.bfloat16
FP8 = mybir.dt.float8e4
AF = mybir.ActivationFunctionType
OP = mybir.AluOpType
AX = mybir.AxisListType

B, M, W, R, IN = 128, 512, 128, 4, 1024
NCORES = 8
BL = B // NCORES            # 16 items per core
DELTA = 1e-6
NCH = M // 128              # 4 chunks of the slot dim
KIN = 1152                  # padded contraction dim (1024 + bias row -> 9*128)
DTOT = 919
NS = 34                     # per-item scalar table width
LSC = 512.0                 # fp8 pre-scale for link/rw
LDS = float(2.0 ** -18)     # rescale after fp8 link matmuls

_dims = dict(rk=R * W, rs=R, wk=W, ws=1, ev=W, wv=W, fg=R, ag=1, wg=1, rm=3 * R)
_ofs = {}
_o = 0
for _n, _d in _dims.items():
    _ofs[_n] = _o
    _o += _d
assert _o == DTOT

# scalar-table columns
S_FG, S_RS, S_WS, S_AG, S_WG = 0, 4, 8, 9, 10
S_M0, S_M1, S_M2 = 11, 15, 19
S_BNR, S_BW, S_RKWV, S_C3, S_OMAG = 23, 27, 28, 32, 33


def _emit(nc, tc, ctx, d):
    STOP = int(os.environ.get("KSTOP", "9"))

    P = ctx.enter_context(tc.tile_pool(name="persist", bufs=1))
    ps = ctx.enter_context(tc.tile_pool(name="ps", bufs=4, space=bass.MemorySpace.PSUM))
    psb = ctx.enter_context(tc.tile_pool(name="psb", bufs=4, space=bass.MemorySpace.PSUM))
    stg = ctx.enter_context(tc.tile_pool(name="stg", bufs=2))
    memp = ctx.enter_context(tc.tile_pool(name="memp", bufs=2))
    mtp = ctx.enter_context(tc.tile_pool(name="mtp", bufs=2))
    lp = ctx.enter_context(tc.tile_pool(name="lp", bufs=8))
    urp = ctx.enter_context(tc.tile_pool(name="urp", bufs=4))
    scr = ctx.enter_context(tc.tile_pool(name="scr", bufs=4))

    def pst(pr, fr, pool=ps):
        return pool.tile([pr, fr], F32, tag="pst", name="pst")

    _uid = [0]

    def sct(fr=NCH * BL, dt=F32):
        # m-part scratch [128, fr] with a unique persistent allocation
        _uid[0] += 1
        return P.tile([128, fr], dt, tag=f"mp{_uid[0]}", name=f"mp{_uid[0]}")

    gates = P.tile([BL, DTOT], F32, tag="gates")

    def gsl(name, a=0, b=None):
        o = _ofs[name]
        if b is None:
            b = _dims[name]
        return gates[:, o + a:o + b]

    def _bail(lvl, t=None):
        if STOP <= lvl:
            if t is None:
                t = gates[:, 0:R * W]
            nc.sync.dma_start(d["out"][:], t)
            return True
        return False

    # view helpers (m-part free layouts)
    def vci(t):
        return t[:].rearrange("p (c i) -> p c i", i=BL)

    def vcir(t, k=R):
        return t[:].rearrange("p (c i k) -> p c i k", i=BL, k=k)

    # ---------------- constants + inputs resident in SBUF ----------------
    consts = P.tile([128, 129], F32, tag="consts")
    nc.sync.dma_start(consts[:], d["consts"][:])
    I128 = consts[:, 0:128]

    def ptrans(out_psum, in_sb):
        p = in_sb.shape[0]
        nc.tensor.transpose(out_psum, in_sb, I128[0:p, 0:p])

    RWT = P.tile([128, NCH * BL * R], F32, tag="RWT")    # rw^T [p,(c,i,r)]
    nc.sync.dma_start(RWT[:].rearrange("p (c j) -> p c j", c=NCH), d["rwt"][:].rearrange("(c p) j -> p c j", p=128))
    PRC = P.tile([128, NCH * BL], F32, tag="PRC")        # precedence^T
    nc.sync.dma_start(PRC[:].rearrange("p (c i) -> p c i", c=NCH), d["prct"][:].rearrange("(c p) i -> p c i", p=128))
    WWI = P.tile([128, NCH * BL], F32, tag="WWI")        # write_weights in ^T
    nc.sync.dma_start(WWI[:].rearrange("p (c i) -> p c i", c=NCH), d["wwt"][:].rearrange("(c p) i -> p c i", p=128))
    USG = P.tile([128, NCH * BL], F32, tag="USG")        # usage in ^T
    nc.sync.dma_start(USG[:].rearrange("p (c i) -> p c i", c=NCH), d["usgt"][:].rearrange("(c p) i -> p c i", p=128))
    DGT = P.tile([128, NCH * BL], F32, tag="DGT")        # diag(L) ^T
    nc.sync.dma_start(DGT[:].rearrange("p (c i) -> p c i", c=NCH), d["diagt"][:].rearrange("(c p) i -> p c i", p=128))

    xta = P.tile([128, 9, BL], BF16, tag="xta")
    nc.sync.dma_start(xta[:], d["xta"][:].rearrange("(k p) i -> p k i", p=128))

    # ---------------- phase A: fused linear + gates (item-part) ----------------
    zps = pst(BL, 512)
    zps2 = pst(BL, DTOT - 512)
    with tc.tile_pool(name="wstream", bufs=4) as wp:
        for k in range(9):
            wk_t = wp.tile([128, DTOT], BF16, tag="w")
            nc.sync.dma_start(wk_t[:], d["wta"][128 * k:128 * (k + 1), :])
            nc.tensor.matmul(zps[:], xta[:, k, :], wk_t[:, 0:512],
                             start=(k == 0), stop=(k == 8))
            nc.tensor.matmul(zps2[:], xta[:, k, :], wk_t[:, 512:DTOT],
                             start=(k == 0), stop=(k == 8))

    def zsl(a, b):
        if b <= 512:
            return zps[:, a:b]
        assert a >= 512
        return zps2[:, a - 512:b - 512]

    nc.scalar.activation(gates[:, 901:907], zsl(901, 907), AF.Sigmoid)           # fg,ag,wg
    nc.scalar.activation(gates[:, 645:773], zsl(645, 773), AF.Sigmoid)           # ev
    nc.scalar.activation(gates[:, 0:512], zsl(0, 512), AF.Tanh)                  # rk
    nc.scalar.activation(gates[:, 516:644], zsl(516, 644), AF.Tanh)              # wk
    nc.scalar.activation(gates[:, 773:901], zsl(773, 901), AF.Tanh)              # wv
    nc.scalar.activation(gates[:, 907:919], zsl(907, 919), AF.Identity)          # rm logits
    # softplus(z) = relu(z) + ln(1 + exp(-|z|)), grouped by activation function
    spts = {}
    for (a, b) in [(512, 516), (644, 645)]:
        spts[a] = scr.tile([BL, b - a], F32, tag="sp", name="spt")
        nc.scalar.activation(spts[a][:], zsl(a, b), AF.Abs)
    for (a, b) in [(512, 516), (644, 645)]:
        nc.scalar.activation(spts[a][:], spts[a][:], AF.Exp, scale=-1.0)
    for (a, b) in [(512, 516), (644, 645)]:
        nc.scalar.activation(spts[a][:], spts[a][:], AF.Ln, bias=1.0)
    for (a, b) in [(512, 516), (644, 645)]:
        nc.scalar.activation(gates[:, a:b], zsl(a, b), AF.Relu)
    for (a, b) in [(512, 516), (644, 645)]:
        nc.vector.tensor_tensor(gates[:, a:b], gates[:, a:b], spts[a][:], op=OP.add)

    # read-mode softmax over groups of 3
    rmz = gates[:, 907:919].rearrange("i (r k) -> i r k", k=3)
    negmax3 = P.tile([BL, R], F32, tag="negmax3")
    nc.vector.tensor_reduce(negmax3[:], rmz, axis=AX.X, op=OP.max, negate=True)
    rme = P.tile([BL, 3 * R], F32, tag="rme")
    nc.vector.tensor_tensor(rme[:].rearrange("i (r k) -> i r k", k=3), rmz,
                            negmax3[:].rearrange("i (r o) -> i r o", o=1).broadcast_to([BL, R, 3]),
                            op=OP.add)
    nc.scalar.activation(rme[:], rme[:], AF.Exp)
    rmsum = P.tile([BL, R], F32, tag="rmsum")
    nc.vector.tensor_reduce(rmsum[:], rme[:].rearrange("i (r k) -> i r k", k=3), axis=AX.X, op=OP.add)
    nc.vector.reciprocal(rmsum[:], rmsum[:])
    rm = P.tile([BL, 3 * R], F32, tag="rm")
    nc.vector.tensor_tensor(rm[:].rearrange("i (r k) -> i r k", k=3),
                            rme[:].rearrange("i (r k) -> i r k", k=3),
                            rmsum[:].rearrange("i (r o) -> i r o", o=1).broadcast_to([BL, R, 3]),
                            op=OP.mult)

    # per-item key-norm scalars (item-part)
    scw = P.tile([BL, W], F32, tag="scw")
    bw128 = P.tile([BL, 1], F32, tag="bw128")
    nc.scalar.activation(scw[:], gsl("wk"), AF.Square, accum_out=bw128[:])
    nc.scalar.activation(bw128[:], bw128[:], AF.Sqrt)
    nc.any.tensor_scalar(bw128[:], bw128[:], float(W), float(W) * DELTA, op0=OP.mult, op1=OP.add)
    bnr = P.tile([BL, R], F32, tag="bnr")
    rkwv = P.tile([BL, R], F32, tag="rkwv")
    for r in range(R):
        nc.scalar.activation(scw[:], gsl("rk", r * W, (r + 1) * W), AF.Square, accum_out=bnr[:, r:r + 1])
        nc.vector.tensor_tensor(scw[:], gsl("rk", r * W, (r + 1) * W), gsl("wv"), op=OP.mult)
        nc.vector.tensor_scalar(scw[:], scw[:], 1.0, None, op0=OP.mult, op1=OP.add,
                                accum_out=rkwv[:, r:r + 1])
    nc.scalar.activation(bnr[:], bnr[:], AF.Sqrt)
    nc.any.tensor_scalar(bnr[:], bnr[:], float(W), float(W) * DELTA, op0=OP.mult, op1=OP.add)
    c3 = P.tile([BL, 1], F32, tag="c3")
    nc.scalar.activation(scw[:], gsl("wv"), AF.Square, accum_out=c3[:])

    # KCM/NRM lhsT tables (w on partitions), f32 then cast to bf16
    KCMf = P.tile([128, BL * 11], F32, tag="KCMf")
    NRMf = P.tile([128, BL * 3], F32, tag="NRMf")
    EVT = P.tile([128, BL], F32, tag="EVT")

    def kcm_col(j):
        return KCMf[:].rearrange("p (i k) -> p i k", k=11)[:, :, j]

    gtp = pst(128, BL, psb)
    ptrans(gtp[:], gsl("wk"))
    nc.any.tensor_copy(kcm_col(0), gtp[:])
    gtp = pst(128, BL, psb)
    ptrans(gtp[:], gsl("ev"))
    nc.any.tensor_copy(EVT[:], gtp[:])
    gtp = pst(128, BL, psb)
    ptrans(gtp[:], gsl("wv"))
    nc.any.tensor_copy(kcm_col(9), gtp[:])
    nc.vector.tensor_tensor(kcm_col(10), kcm_col(9), EVT[:], op=OP.mult)  # ev*wv
    for r in range(R):
        gtp = pst(128, BL, psb)
        ptrans(gtp[:], gsl("rk", r * W, (r + 1) * W))
        nc.any.tensor_copy(kcm_col(1 + r), gtp[:])
        nc.vector.tensor_tensor(kcm_col(5 + r), kcm_col(1 + r), EVT[:], op=OP.mult)
    nrm3 = NRMf[:].rearrange("p (i k) -> p i k", k=3)
    nc.any.memset(nrm3[:, :, 0], 1.0)
    nc.any.tensor_copy(nrm3[:, :, 1], EVT[:])
    nc.scalar.activation(nrm3[:, :, 2], EVT[:], AF.Square)
    KCMb = P.tile([128, BL * 11], BF16, tag="KCMb")
    nc.any.tensor_copy(KCMb[:], KCMf[:])
    NRMb = P.tile([128, BL * 3], BF16, tag="NRMb")
    nc.any.tensor_copy(NRMb[:], NRMf[:])

    # ---------------- scalar table -> partition-broadcast SCB ----------------
    SCAL = P.tile([BL, NS], F32, tag="SCAL")
    nc.any.tensor_copy(SCAL[:, S_FG:S_FG + R], gsl("fg"))
    nc.any.tensor_copy(SCAL[:, S_RS:S_RS + R], gsl("rs"))
    nc.any.tensor_copy(SCAL[:, S_WS:S_WS + 1], gsl("ws"))
    nc.any.tensor_copy(SCAL[:, S_AG:S_AG + 1], gsl("ag"))
    nc.any.tensor_copy(SCAL[:, S_WG:S_WG + 1], gsl("wg"))
    for k in range(3):
        nc.any.tensor_copy(SCAL[:, S_M0 + R * k:S_M0 + R * (k + 1)],
                           rm[:].rearrange("i (r k) -> i r k", k=3)[:, :, k])
    nc.any.tensor_copy(SCAL[:, S_BNR:S_BNR + R], bnr[:])
    nc.any.tensor_copy(SCAL[:, S_BW:S_BW + 1], bw128[:])
    nc.any.tensor_copy(SCAL[:, S_RKWV:S_RKWV + R], rkwv[:])
    nc.any.tensor_copy(SCAL[:, S_C3:S_C3 + 1], c3[:])
    nc.any.tensor_scalar(SCAL[:, S_OMAG:S_OMAG + 1], gsl("ag"), -1.0, 1.0,
                         op0=OP.mult, op1=OP.add)
    scalrow = P.tile([1, BL * NS], F32, tag="scalrow")
    nc.scalar.dma_start(scalrow[:], SCAL[:])
    SCB = P.tile([128, BL * NS], F32, tag="SCB")
    nc.gpsimd.partition_broadcast(SCB[:], scalrow[:])

    def scb(s, w=1):
        # [128, BL, w] view of scalar cols s..s+w
        return SCB[:].rearrange("p (i s) -> p i s", s=NS)[:, :, s:s + w]

    def scb_c(s):
        # broadcast over chunks -> [128, NCH, BL]
        return SCB[:].rearrange("p (o i s) -> p o i s", o=1, s=NS)[:, :, :, s] \
            .broadcast_to([128, NCH, BL])

    def scb_cr(s):
        # per-(i,r) scalars broadcast over chunks -> [128, NCH, BL, R]
        return SCB[:].rearrange("p (o i s) -> p o i s", o=1, s=NS)[:, :, :, s:s + R] \
            .broadcast_to([128, NCH, BL, R])

    def bc_r(t):
        # [128, NCH*BL] -> [128, NCH, BL, R] broadcast over r
        return t[:].rearrange("p (c i o) -> p c i o", i=BL, o=1).broadcast_to([128, NCH, BL, R])

    if _bail(1):
        return

    # ---------------- phase A2: usage / u / log u (m-part) ----------------
    psi4 = sct(NCH * BL * R)
    nc.vector.tensor_tensor(vcir(psi4), vcir(RWT), scb_cr(S_FG), op=OP.mult)
    nc.any.tensor_scalar(psi4[:], psi4[:], 1.0, None, op0=OP.subtract)  # fg*rw - 1
    psi = sct()
    p4 = vcir(psi4)
    nc.vector.tensor_tensor(vci(psi), p4[:, :, :, 0], p4[:, :, :, 1], op=OP.mult)
    nc.vector.tensor_tensor(p4[:, :, :, 2], p4[:, :, :, 2], p4[:, :, :, 3], op=OP.mult)
    nc.vector.tensor_tensor(vci(psi), vci(psi), p4[:, :, :, 2], op=OP.mult)

    u_sb = P.tile([128, NCH * BL], F32, tag="u_sb")
    nc.vector.tensor_tensor(u_sb[:], USG[:], WWI[:], op=OP.mult)
    nc.vector.tensor_tensor(u_sb[:], USG[:], u_sb[:], op=OP.subtract)
    nc.vector.tensor_tensor(u_sb[:], u_sb[:], WWI[:], op=OP.add)
    nc.vector.tensor_tensor(u_sb[:], u_sb[:], psi[:], op=OP.mult)
    nc.any.tensor_scalar(u_sb[:], u_sb[:], 1.0 - DELTA, DELTA, op0=OP.mult, op1=OP.add)

    # u_ip (item-part copy of u) -> u_flat single row for mask broadcasts
    uips = pst(BL, M, psb)
    for c in range(NCH):
        ptrans(uips[:, 128 * c:128 * (c + 1)], vci(u_sb)[:, c, :])
    u_ip = P.tile([BL, M], F32, tag="u_ip")
    nc.any.tensor_copy(u_ip[:], uips[:])
    u_flat = P.tile([1, BL * M], F32, tag="u_flat")
    for j in range(4):
        nc.scalar.dma_start(u_flat[0:1, 2048 * j:2048 * (j + 1)], u_ip[4 * j:4 * (j + 1), :])

    if _bail(2, u_ip[:, 0:R * W]):
        return

    # ---------------- phase B + alloc mask (per item) ----------------
    BCOLL = P.tile([128, NCH * BL * 64], BF16, tag="BCOLL")
    S_acc = P.tile([128, NCH * BL], F32, tag="S_acc")
    stg2 = None
    for i in range(BL):
        if i % 2 == 0:
            mem2 = memp.tile([128, 2, M], BF16, tag="memt")
            nc.sync.dma_start(mem2[:], d["memt"][i:i + 2].rearrange("b w m -> w b m"))
            stg2 = stg.tile([128, M], BF16, tag="stg2", name="stgB")
            nc.gpsimd.memset(stg2[:], 0.0)
        mti = mem2[:, i % 2, :]
        mt2 = mtp.tile([128, M], BF16, tag="mt2")
        nc.gpsimd.tensor_tensor(mt2[:], mti, mti, op=OP.mult)
        kb_ps = pst(11, M)
        nc.tensor.matmul(kb_ps[:], KCMb[:, 11 * i:11 * (i + 1)], mti, start=True, stop=True)
        nr_ps = pst(3, M)
        nc.tensor.matmul(nr_ps[:], NRMb[:, 3 * i:3 * (i + 1)], mt2[:], start=True, stop=True)
        o = 64 * (i % 2)
        nc.any.tensor_copy(stg2[o:o + 11, :], kb_ps[:])
        nc.any.tensor_copy(stg2[o + 32:o + 35, :], nr_ps[:])
        if i % 2 == 1:
            dst = BCOLL[:].rearrange("p (c i k) -> p c (i k)", i=BL, k=64)[:, :, 64 * (i - 1):64 * (i + 1)]
            nc.sync.dma_start_transpose(dst, stg2[:])
        # --- mask for item i ---
        urep = urp.tile([128, M], F32, tag="urep")
        nc.gpsimd.partition_broadcast(urep[:], u_flat[0:1, M * i:M * (i + 1)])
        lurep = urp.tile([128, M], F32, tag="lurep")
        nc.scalar.activation(lurep[:], urep[:], AF.Ln)
        mscr = scr.tile([128, M], F32, tag="mscr", name="mscr")
        for c in range(NCH):
            nc.vector.scalar_tensor_tensor(mscr[:], urep[:], vci(u_sb)[:, c, i:i + 1],
                                           lurep[:], op0=OP.is_lt, op1=OP.mult,
                                           accum_out=vci(S_acc)[:, c, i:i + 1])

    if _bail(3, u_ip[:, 0:R * W]):
        return

    # ---------------- phase C: batched m-part weights ----------------
    bcf = vcir(BCOLL, 64)

    def bcol(j):
        return bcf[:, :, :, j]

    WCN, T1, T2, S0, S1, S2 = bcol(0), bcol(9), bcol(10), bcol(32), bcol(33), bcol(34)

    # alloc = (1-u) * exp(S)
    EXS = sct()
    nc.scalar.activation(EXS[:], S_acc[:], AF.Exp)
    onemu = sct()
    nc.any.tensor_scalar(onemu[:], u_sb[:], -1.0, 1.0, op0=OP.mult, op1=OP.add)
    alloc = sct()
    nc.vector.tensor_tensor(alloc[:], onemu[:], EXS[:], op=OP.mult)

    # write content weights wcw (no max subtraction; logits are tiny)
    AO = sct()
    nc.scalar.activation(vci(AO), S0, AF.Sqrt)
    nc.any.tensor_scalar(AO[:], AO[:], 1.0, DELTA, op0=OP.mult, op1=OP.add)
    wden = sct()
    nc.vector.tensor_tensor(vci(wden), vci(AO), scb_c(S_BW), op=OP.mult)
    nc.any.tensor_scalar(wden[:], wden[:], 1.0, DELTA, op0=OP.mult, op1=OP.add)
    nc.vector.reciprocal(wden[:], wden[:])
    wz = sct()
    nc.vector.tensor_tensor(vci(wz), WCN, vci(wden), op=OP.mult)
    nc.vector.tensor_tensor(vci(wz), vci(wz), scb_c(S_WS), op=OP.mult)
    nc.scalar.activation(wz[:], wz[:], AF.Exp)
    wzs = P.tile([128, BL], F32, tag="wzs")
    nc.vector.tensor_reduce(wzs[:], wz[:].rearrange("p (c i) -> p i c", i=BL), axis=AX.X, op=OP.add)
    WZS = P.tile([128, BL], F32, tag="WZS")
    nc.gpsimd.partition_all_reduce(WZS[:], wzs[:], channels=128, reduce_op=bass_isa.ReduceOp.add)
    nc.vector.reciprocal(WZS[:], WZS[:])
    wcw = sct()
    nc.vector.tensor_tensor(vci(wcw), vci(wz),
                            WZS[:].rearrange("p (o i) -> p o i", o=1).broadcast_to([128, NCH, BL]),
                            op=OP.mult)

    # write weights ww = wg * (ag*alloc + (1-ag)*wcw)
    ww = P.tile([128, NCH * BL], F32, tag="ww")
    nc.vector.tensor_tensor(vci(alloc), vci(alloc), scb_c(S_AG), op=OP.mult)
    nc.vector.tensor_tensor(vci(ww), vci(wcw), scb_c(S_OMAG), op=OP.mult)
    nc.vector.tensor_tensor(ww[:], ww[:], alloc[:], op=OP.add)
    nc.vector.tensor_tensor(vci(ww), vci(ww), scb_c(S_WG), op=OP.mult)

    if _bail(4, u_ip[:, 0:R * W]):
        return

    # fp8 lhsT combos for link matmuls: [rw*512 | rw*512*ww]
    RWC8 = P.tile([128, NCH * BL * 8], FP8, tag="RWC8")
    rwc = vcir(RWC8, 8)
    nc.any.tensor_scalar(rwc[:, :, :, 0:4], vcir(RWT), LSC, None, op0=OP.mult)
    nc.vector.scalar_tensor_tensor(rwc[:, :, :, 4:8], vcir(RWT), LSC, bc_r(ww),
                                   op0=OP.mult, op1=OP.mult)

    if _bail(5, u_ip[:, 0:R * W]):
        return

    # ---------------- phase D: link products (per item) ----------------
    DCOLL = P.tile([128, NCH * BL * 64], BF16, tag="DCOLL")
    stg2d = None
    for i in range(BL):
        ll = lp.tile([128, 2, NCH, M], FP8, tag="ll")
        nc.sync.dma_start(ll[:], d["llt"][i].rearrange("g (c p) n -> p g c n", p=128))
        if i % 2 == 0:
            stg2d = stg.tile([128, M], BF16, tag="stg2", name="stgD")
            nc.gpsimd.memset(stg2d[:], 0.0)
        bps = pst(8, M)
        fps = pst(8, M)
        for c in range(NCH):
            lhs = RWC8[:].rearrange("p (c i k) -> p c i k", i=BL, k=8)[:, c, i, :]
            nc.tensor.matmul(bps[:], lhs, ll[:, 0, c, :], start=(c == 0), stop=(c == NCH - 1))
            nc.tensor.matmul(fps[:], lhs, ll[:, 1, c, :], start=(c == 0), stop=(c == NCH - 1))
        o = 64 * (i % 2)
        nc.any.tensor_scalar(stg2d[o:o + 8, :], bps[:], LDS, None, op0=OP.mult)
        nc.any.tensor_scalar(stg2d[o + 32:o + 40, :], fps[:], LDS, None, op0=OP.mult)
        if i % 2 == 1:
            dst = DCOLL[:].rearrange("p (c i k) -> p c (i k)", i=BL, k=64)[:, :, 64 * (i - 1):64 * (i + 1)]
            nc.sync.dma_start_transpose(dst, stg2d[:])

    if _bail(6, u_ip[:, 0:R * W]):
        return

    # new-memory norms AN
    ww2 = sct()
    nc.scalar.activation(ww2[:], ww[:], AF.Square)
    q1 = sct()
    nc.vector.tensor_tensor(vci(q1), S1, T1, op=OP.subtract)
    nc.vector.tensor_tensor(q1[:], ww[:], q1[:], op=OP.mult)
    q2 = sct()
    nc.any.tensor_scalar(vci(q2), T2, -2.0, None, op0=OP.mult)
    nc.vector.tensor_tensor(vci(q2), vci(q2), S2, op=OP.add)
    nc.vector.tensor_tensor(vci(q2), vci(q2), scb_c(S_C3), op=OP.add)
    nc.vector.tensor_tensor(q2[:], q2[:], ww2[:], op=OP.mult)
    AN = sct()
    nc.any.tensor_scalar(q1[:], q1[:], -2.0, None, op0=OP.mult)
    nc.vector.tensor_tensor(vci(AN), S0, vci(q1), op=OP.add)
    nc.vector.tensor_tensor(AN[:], AN[:], q2[:], op=OP.add)
    nc.scalar.activation(AN[:], AN[:], AF.Sqrt)
    nc.any.tensor_scalar(AN[:], AN[:], 1.0, DELTA, op0=OP.mult, op1=OP.add)

    # read content weights cw (scaled by mode2 / csum)
    cnum = sct(NCH * BL * R)
    cn = vcir(cnum)
    cwA = bcf[:, :, :, 1:5]
    cwB = bcf[:, :, :, 5:9]  # bf16 views read directly
    nc.vector.tensor_tensor(cn, cwB, bc_r(ww), op=OP.mult)
    nc.vector.tensor_tensor(cn, cwA, cn, op=OP.subtract)
    ct = sct(NCH * BL * R)
    nc.vector.tensor_tensor(vcir(ct), bc_r(ww), scb_cr(S_RKWV), op=OP.mult)
    nc.vector.tensor_tensor(cnum[:], cnum[:], ct[:], op=OP.add)
    cden = sct(NCH * BL * R)
    nc.vector.tensor_tensor(vcir(cden), bc_r(AN), scb_cr(S_BNR), op=OP.mult)
    nc.any.tensor_scalar(cden[:], cden[:], 1.0, DELTA, op0=OP.mult, op1=OP.add)
    nc.vector.reciprocal(cden[:], cden[:])
    nc.vector.tensor_tensor(cnum[:], cnum[:], cden[:], op=OP.mult)
    nc.vector.tensor_tensor(cn, cn, scb_cr(S_RS), op=OP.mult)
    nc.scalar.activation(cnum[:], cnum[:], AF.Exp)
    csum = P.tile([128, BL * R], F32, tag="csum")
    nc.vector.tensor_reduce(csum[:], cnum[:].rearrange("p (c j) -> p j c", j=BL * R),
                            axis=AX.X, op=OP.add)
    CSR = P.tile([128, BL * R], F32, tag="CSR")
    nc.gpsimd.partition_all_reduce(CSR[:], csum[:], channels=128, reduce_op=bass_isa.ReduceOp.add)
    nc.vector.reciprocal(CSR[:], CSR[:])
    nc.vector.tensor_tensor(CSR[:].rearrange("p (i r) -> p i r", r=R),
                            CSR[:].rearrange("p (i r) -> p i r", r=R),
                            scb(S_M2, R), op=OP.mult)
    cwm2 = sct(NCH * BL * R)   # mode2 * cw
    nc.vector.tensor_tensor(vcir(cwm2), cn,
                            CSR[:].rearrange("p (o i r) -> p o i r", o=1, r=R)
                            .broadcast_to([128, NCH, BL, R]), op=OP.mult)


    # ---------------- phase E: assemble fwd/bwd/rw_new (m-part) ----------------
    dcv = vcir(DCOLL, 64)
    P1, P2 = dcv[:, :, :, 0:4], dcv[:, :, :, 4:8]
    F1, F2 = dcv[:, :, :, 32:36], dcv[:, :, :, 36:40]

    # cpr = prec . rw_r ; dwr = rw_r . ww   (per item, read head)
    scr4 = sct(NCH * BL * R)
    prcb = PRC[:].rearrange("p (c i o) -> p c i o", i=BL, o=1).broadcast_to([128, NCH, BL, R])
    nc.vector.tensor_tensor(vcir(scr4), vcir(RWT), prcb, op=OP.mult)
    CDW = P.tile([128, 2 * BL * R], F32, tag="CDW")
    nc.vector.tensor_reduce(CDW[:, 0:BL * R], scr4[:].rearrange("p (c j) -> p j c", j=BL * R),
                            axis=AX.X, op=OP.add)
    scr4b = sct(NCH * BL * R)
    nc.vector.tensor_tensor(vcir(scr4b), vcir(RWT), bc_r(ww), op=OP.mult)
    nc.vector.tensor_reduce(CDW[:, BL * R:2 * BL * R],
                            scr4b[:].rearrange("p (c j) -> p j c", j=BL * R),
                            axis=AX.X, op=OP.add)
    CDWr = P.tile([128, 2 * BL * R], F32, tag="CDWr")
    nc.gpsimd.partition_all_reduce(CDWr[:], CDW[:], channels=128, reduce_op=bass_isa.ReduceOp.add)

    def cdw_b(off):
        return CDWr[:, off:off + BL * R].rearrange("p (o i r) -> p o i r", o=1, r=R) \
            .broadcast_to([128, NCH, BL, R])

    # dv = (1-2ww)*diag + ww*prec ; DR = rw * dv
    dv = sct()
    nc.any.tensor_scalar(dv[:], ww[:], -2.0, 1.0, op0=OP.mult, op1=OP.add)
    nc.vector.tensor_tensor(dv[:], dv[:], DGT[:], op=OP.mult)
    t2m = sct()
    nc.vector.tensor_tensor(t2m[:], ww[:], PRC[:], op=OP.mult)
    nc.vector.tensor_tensor(dv[:], dv[:], t2m[:], op=OP.add)
    DR = sct(NCH * BL * R)
    nc.vector.tensor_tensor(vcir(DR), vcir(RWT), bc_r(dv), op=OP.mult)

    onemw = sct()
    nc.any.tensor_scalar(onemw[:], ww[:], -1.0, 1.0, op0=OP.mult, op1=OP.add)

    # fwd = F1*(1-ww) - F2 + ww (x) cpr - DR   (then scaled by mode1)
    fwd = sct(NCH * BL * R)
    fv = vcir(fwd)
    nc.vector.tensor_tensor(fv, F1, bc_r(onemw), op=OP.mult)
    nc.vector.tensor_tensor(fv, fv, F2, op=OP.subtract)
    ftt = sct(NCH * BL * R)
    nc.vector.tensor_tensor(vcir(ftt), bc_r(ww), cdw_b(0), op=OP.mult)
    nc.vector.tensor_tensor(fwd[:], fwd[:], ftt[:], op=OP.add)
    nc.vector.tensor_tensor(fwd[:], fwd[:], DR[:], op=OP.subtract)

    # bwd = P1*(1-ww) - P2 + prec (x) dwr - DR  (then scaled by mode0)
    bwd = sct(NCH * BL * R)
    bv = vcir(bwd)
    nc.vector.tensor_tensor(bv, P1, bc_r(onemw), op=OP.mult)
    nc.vector.tensor_tensor(bv, bv, P2, op=OP.subtract)
    nc.vector.tensor_tensor(vcir(ftt), prcb, cdw_b(BL * R), op=OP.mult)
    nc.vector.tensor_tensor(bwd[:], bwd[:], ftt[:], op=OP.add)
    nc.vector.tensor_tensor(bwd[:], bwd[:], DR[:], op=OP.subtract)

    rwnew = P.tile([128, NCH * BL * R], F32, tag="rwnew")
    nc.vector.tensor_tensor(bv, bv, scb_cr(S_M0), op=OP.mult)
    nc.vector.tensor_tensor(fv, fv, scb_cr(S_M1), op=OP.mult)
    nc.vector.tensor_tensor(rwnew[:], bwd[:], fwd[:], op=OP.add)
    nc.vector.tensor_tensor(rwnew[:], rwnew[:], cwm2[:], op=OP.add)

    # sc = rwnew . ww
    nc.vector.tensor_tensor(vcir(scr4), vcir(rwnew), bc_r(ww), op=OP.mult)
    SC1 = P.tile([128, BL * R], F32, tag="SC1")
    nc.vector.tensor_reduce(SC1[:], scr4[:].rearrange("p (c j) -> p j c", j=BL * R),
                            axis=AX.X, op=OP.add)
    SCR_ = P.tile([128, BL * R], F32, tag="SCR_")
    nc.gpsimd.partition_all_reduce(SCR_[:], SC1[:], channels=128, reduce_op=bass_isa.ReduceOp.add)

    # bf16 lhsT for read vectors: [rwnew | rwnew*ww]
    RVL = P.tile([128, NCH * BL * 8], BF16, tag="RVL")
    rvv = vcir(RVL, 8)
    nc.any.tensor_copy(rvv[:, :, :, 0:4], vcir(rwnew))
    nc.vector.tensor_tensor(rvv[:, :, :, 4:8], vcir(rwnew), bc_r(ww), op=OP.mult)

    if _bail(7, u_ip[:, 0:R * W]):
        return

    # ---------------- phase F: read vectors (per item) ----------------
    rv16 = P.tile([BL, 2 * R * W], F32, tag="rv16")
    for i in range(BL):
        if i % 2 == 0:
            mn2 = memp.tile([128, 2, NCH, W], BF16, tag="memn")
            nc.sync.dma_start(mn2[:], d["memn"][i:i + 2].rearrange("b (c p) w -> p b c w", p=128))
        rvp = pst(8, W)
        for c in range(NCH):
            nc.tensor.matmul(rvp[:], rvv[:, c, i, :], mn2[:, i % 2, c, :],
                             start=(c == 0), stop=(c == NCH - 1))
        rvs = stg.tile([8, W], F32, tag="rvs", name="rvs")
        nc.any.tensor_copy(rvs[:], rvp[:])
        nc.scalar.dma_start(rv16[i:i + 1, :], rvs[:])

    if _bail(8, rv16[:, 0:R * W]):
        return

    # ---------------- phase G: final combine (item-part) ----------------
    sc_ip = P.tile([BL, R], F32, tag="sc_ip")
    nc.sync.dma_start(sc_ip[:], SCR_[0:1, :])

    def g4(t, n=W):
        return t.rearrange("i (r m) -> i r m", m=n)

    def bcm_w(t):
        return t.rearrange("i (o m) -> i o m", o=1).broadcast_to([BL, R, W])

    def bc4_w(t):
        return t.rearrange("i (r o) -> i r o", o=1).broadcast_to([BL, R, W])

    RA = g4(rv16[:, 0:R * W])
    RB = g4(rv16[:, R * W:2 * R * W])
    out_sb = P.tile([BL, R * W], F32, tag="out_sb")
    nc.vector.tensor_tensor(g4(out_sb[:]), RB, bcm_w(gsl("ev")), op=OP.mult)
    nc.vector.tensor_tensor(g4(out_sb[:]), RA, g4(out_sb[:]), op=OP.subtract)
    wvt = P.tile([BL, R * W], F32, tag="wvt")
    nc.vector.tensor_tensor(g4(wvt[:]), bcm_w(gsl("wv")), bc4_w(sc_ip[:]), op=OP.mult)
    nc.vector.tensor_tensor(out_sb[:], out_sb[:], wvt[:], op=OP.add)
    nc.sync.dma_start(d["out"][:], out_sb[:])


_NC_CACHE = {}


def build_nc():
    if "nc" in _NC_CACHE:
        return _NC_CACHE["nc"]
    nc = bacc.Bacc("TRN2", target_bir_lowering=False, debug=False)
    d = {}
    d["consts"] = nc.dram_tensor("consts", [128, 129], F32, kind="ExternalInput")
    d["xta"] = nc.dram_tensor("xta", [KIN, BL], BF16, kind="ExternalInput")
    d["wta"] = nc.dram_tensor("wta", [KIN, DTOT], BF16, kind="ExternalInput")
    d["memt"] = nc.dram_tensor("memt", [BL, W, M], BF16, kind="ExternalInput")
    d["memn"] = nc.dram_tensor("memn", [BL, M, W], BF16, kind="ExternalInput")
    d["llt"] = nc.dram_tensor("llt", [BL, 2, M, M], FP8, kind="ExternalInput")
    d["rwt"] = nc.dram_tensor("rwt", [M, BL * R], F32, kind="ExternalInput")
    d["prct"] = nc.dram_tensor("prct", [M, BL], F32, kind="ExternalInput")
    d["wwt"] = nc.dram_tensor("wwt", [M, BL], F32, kind="ExternalInput")
    d["usgt"] = nc.dram_tensor("usgt", [M, BL], F32, kind="ExternalInput")
    d["diagt"] = nc.dram_tensor("diagt", [M, BL], F32, kind="ExternalInput")
    d["out"] = nc.dram_tensor("out", [BL, R * W], F32, kind="ExternalOutput")
    with tile.TileContext(nc) as tc:
        with ExitStack() as ctx:
            _emit(nc, tc, ctx, d)
    nc.compile()
    _NC_CACHE["nc"] = nc
    return nc


def make_in_maps(inputs):
    names = ["rk", "rs", "wk", "ws", "ev", "wv", "fg", "ag", "wg", "rm"]
    Wall = np.concatenate([np.asarray(inputs[f"W_{n}"]) for n in names], axis=0).astype(np.float32)
    ball = np.concatenate([np.asarray(inputs[f"b_{n}"]) for n in names], axis=0).astype(np.float32)
    wta = np.zeros((KIN, DTOT), np.float32)
    wta[:IN] = Wall.T
    wta[IN] = ball
    wta = wta.astype(ml_dtypes.bfloat16)
    consts = np.zeros((128, 129), np.float32)
    consts[:, :128] = np.eye(128, dtype=np.float32)
    consts[:, 128] = 1.0

    x = np.asarray(inputs["x"], np.float32)
    mem = np.asarray(inputs["memory"], np.float32)
    link = np.asarray(inputs["link_matrix"], np.float32)[:, 0]
    prec = np.asarray(inputs["precedence"], np.float32)[:, 0]
    rw = np.asarray(inputs["read_weights"], np.float32)
    wwin = np.asarray(inputs["write_weights"], np.float32)[:, 0]
    usage = np.asarray(inputs["usage_vector"], np.float32)

    in_maps = []
    for cix in range(NCORES):
        sl = slice(cix * BL, (cix + 1) * BL)
        xta = np.zeros((KIN, BL), np.float32)
        xta[:IN] = x[sl].T
        xta[IN] = 1.0
        rws = rw[sl]
        lk = link[sl]
        llt = np.stack([lk, lk.transpose(0, 2, 1)], axis=1) * LSC
        diag = np.ascontiguousarray(np.diagonal(lk, axis1=1, axis2=2))
        in_maps.append({
            "consts": consts,
            "xta": xta.astype(ml_dtypes.bfloat16),
            "wta": wta,
            "memt": np.ascontiguousarray(mem[sl].transpose(0, 2, 1)).astype(ml_dtypes.bfloat16),
            "memn": np.ascontiguousarray(mem[sl]).astype(ml_dtypes.bfloat16),
            "llt": np.ascontiguousarray(llt).astype(ml_dtypes.float8_e4m3fn),
            "rwt": np.ascontiguousarray(rws.transpose(2, 0, 1).reshape(M, BL * R)),
            "prct": np.ascontiguousarray(prec[sl].T),
            "wwt": np.ascontiguousarray(wwin[sl].T),
            "usgt": np.ascontiguousarray(usage[sl].T),
            "diagt": np.ascontiguousarray(diag.T),
        })
    return in_maps


def kernel(**inputs):
    nc = build_nc()
    in_maps = make_in_maps(inputs)
    res = run_bass_kernel_spmd(nc, in_maps, list(range(NCORES))).results
    out = np.concatenate([res[c]["out"].reshape(BL, R, W) for c in range(NCORES)], axis=0)
    return out.astype(np.float32)


# revision 4
# speedup vs baseline: 1.0409x; 1.0020x over previous
"""Trainium2 Bass kernel v2 for the DNC memory-step problem (nn_DNC_3882650436261).

Pure data-parallel over batch (128 -> 16 items x 8 cores). Same algebraic
decomposition as v1 (sort-free allocation, link products via matmuls against
L / L^T, mem_new never materialized), but with a new hardware mapping:

  - "m-part world": the 512-slot dim lives on partitions (4 chunks of 128);
    all elementwise/softmax math runs at full 128-lane width with free dim
    = (chunk, item[, read-head]).  Softmax sums over slots use a free-dim
    reduce over chunks + gpsimd partition_all_reduce.
  - big matmuls in bf16 (1 cycle/row on PE instead of 4 for fp32); link
    matmuls in fp8e4m3 with inputs pre-scaled by 512 (host) / 512 (rw,
    on-chip) to stay in fp8 normal range; PSUM results rescaled by 2^-18.
  - L and L^T both streamed from HBM (host-prepped, fp8) -> zero on-chip
    128x128 PE transposes of the link matrix.
  - per-item [16,512] item-part matmul results are scattered into m-part
    collectors with single dma_start_transpose (hw XBAR) instructions.
  - the sort-free allocation mask runs as one fused scalar_tensor_tensor
    (compare * log-u, accumulate over free dim) per (item, chunk).

Self-contained: hardcodes all shapes; no file reads.
"""
import os
import numpy as np
import ml_dtypes
from contextlib import ExitStack

import concourse.bass as bass
import concourse.tile as tile
import concourse.bass_isa as bass_isa
from concourse import bacc, mybir
from concourse.bass_utils import run_bass_kernel_spmd

F32 = mybir.dt.float32
BF16 = mybir.dt.bfloat16
FP8 = mybir.dt.float8e4
AF = mybir.ActivationFunctionType
OP = mybir.AluOpType
AX = mybir.AxisListType

B, M, W, R, IN = 128, 512, 128, 4, 1024
NCORES = 8
BL = B // NCORES            # 16 items per core
DELTA = 1e-6
NCH = M // 128              # 4 chunks of the slot dim
KIN = 1152                  # padded contraction dim (1024 + bias row -> 9*128)
DTOT = 919
NS = 34                     # per-item scalar table width
LSC = 512.0                 # fp8 pre-scale for link/rw
LDS = float(2.0 ** -18)     # rescale after fp8 link matmuls

_dims = dict(rk=R * W, rs=R, wk=W, ws=1, ev=W, wv=W, fg=R, ag=1, wg=1, rm=3 * R)
_ofs = {}
_o = 0
for _n, _d in _dims.items():
    _ofs[_n] = _o
    _o += _d
assert _o == DTOT

# scalar-table columns
S_FG, S_RS, S_WS, S_AG, S_WG = 0, 4, 8, 9, 10
S_M0, S_M1, S_M2 = 11, 15, 19
S_BNR, S_BW, S_RKWV, S_C3, S_OMAG = 23, 27, 28, 32, 33


def _emit(nc, tc, ctx, d):
    STOP = int(os.environ.get("KSTOP", "9"))

    P = ctx.enter_context(tc.tile_pool(name="persist", bufs=1))
    ps = ctx.enter_context(tc.tile_pool(name="ps", bufs=4, space=bass.MemorySpace.PSUM))
    psb = ctx.enter_context(tc.tile_pool(name="psb", bufs=4, space=bass.MemorySpace.PSUM))
    stg = ctx.enter_context(tc.tile_pool(name="stg", bufs=2))
    memp = ctx.enter_context(tc.tile_pool(name="memp", bufs=2))
    mtp = ctx.enter_context(tc.tile_pool(name="mtp", bufs=2))
    lp = ctx.enter_context(tc.tile_pool(name="lp", bufs=6))
    urp = ctx.enter_context(tc.tile_pool(name="urp", bufs=8))
    scr = ctx.enter_context(tc.tile_pool(name="scr", bufs=4))

    def pst(pr, fr, pool=ps):
        return pool.tile([pr, fr], F32, tag="pst", name="pst")

    _uid = [0]

    def sct(fr=NCH * BL, dt=F32):
        # m-part scratch [128, fr] with a unique persistent allocation
        _uid[0] += 1
        return P.tile([128, fr], dt, tag=f"mp{_uid[0]}", name=f"mp{_uid[0]}")

    gates = P.tile([BL, DTOT], F32, tag="gates")

    def gsl(name, a=0, b=None):
        o = _ofs[name]
        if b is None:
            b = _dims[name]
        return gates[:, o + a:o + b]

    def _bail(lvl, t=None):
        if STOP <= lvl:
            if t is None:
                t = gates[:, 0:R * W]
            nc.sync.dma_start(d["out"][:], t)
            return True
        return False

    # view helpers (m-part free layouts)
    def vci(t):
        return t[:].rearrange("p (c i) -> p c i", i=BL)

    def vcir(t, k=R):
        return t[:].rearrange("p (c i k) -> p c i k", i=BL, k=k)

    # ---------------- constants + inputs resident in SBUF ----------------
    consts = P.tile([128, 129], F32, tag="consts")
    nc.sync.dma_start(consts[:], d["consts"][:])
    I128 = consts[:, 0:128]

    def ptrans(out_psum, in_sb):
        p = in_sb.shape[0]
        nc.tensor.transpose(out_psum, in_sb, I128[0:p, 0:p])


    xta = P.tile([128, 9, BL], BF16, tag="xta")
    nc.sync.dma_start(xta[:], d["xta"][:].rearrange("(k p) i -> p k i", p=128))

    # ---------------- phase A: fused linear + gates (item-part) ----------------
    zps = pst(BL, 512)
    zps2 = pst(BL, DTOT - 512)
    zpf = pst(BL, 6, psb)     # priority: fg/ag/wg columns (901:907)
    with tc.tile_pool(name="wstream", bufs=9) as wp:
        wkts = []
        for k in range(9):
            wk_t = wp.tile([128, DTOT], BF16, tag="w")
            nc.sync.dma_start(wk_t[:], d["wta"][128 * k:128 * (k + 1), :])
            wkts.append(wk_t)
            nc.tensor.matmul(zpf[:], xta[:, k, :], wk_t[:, 901:907],
                             start=(k == 0), stop=(k == 8))
        for k in range(9):
            nc.tensor.matmul(zps[:], xta[:, k, :], wkts[k][:, 0:512],
                             start=(k == 0), stop=(k == 8))
            nc.tensor.matmul(zps2[:], xta[:, k, :], wkts[k][:, 512:DTOT],
                             start=(k == 0), stop=(k == 8))

    RWT = P.tile([128, NCH * BL * R], F32, tag="RWT")    # rw^T [p,(c,i,r)]
    nc.sync.dma_start(RWT[:].rearrange("p (c j) -> p c j", c=NCH), d["rwt"][:].rearrange("(c p) j -> p c j", p=128))
    PRC = P.tile([128, NCH * BL], F32, tag="PRC")        # precedence^T
    nc.sync.dma_start(PRC[:].rearrange("p (c i) -> p c i", c=NCH), d["prct"][:].rearrange("(c p) i -> p c i", p=128))
    WWI = P.tile([128, NCH * BL], F32, tag="WWI")        # write_weights in ^T
    nc.sync.dma_start(WWI[:].rearrange("p (c i) -> p c i", c=NCH), d["wwt"][:].rearrange("(c p) i -> p c i", p=128))
    USG = P.tile([128, NCH * BL], F32, tag="USG")        # usage in ^T
    nc.sync.dma_start(USG[:].rearrange("p (c i) -> p c i", c=NCH), d["usgt"][:].rearrange("(c p) i -> p c i", p=128))
    DGT = P.tile([128, NCH * BL], F32, tag="DGT")        # diag(L) ^T
    nc.sync.dma_start(DGT[:].rearrange("p (c i) -> p c i", c=NCH), d["diagt"][:].rearrange("(c p) i -> p c i", p=128))

    def zsl(a, b):
        if b <= 512:
            return zps[:, a:b]
        assert a >= 512
        return zps2[:, a - 512:b - 512]

    nc.scalar.activation(gates[:, 901:907], zpf[:], AF.Sigmoid)           # fg,ag,wg (priority)
    nc.scalar.activation(gates[:, 645:773], zsl(645, 773), AF.Sigmoid)           # ev
    nc.scalar.activation(gates[:, 0:512], zsl(0, 512), AF.Tanh)                  # rk
    nc.scalar.activation(gates[:, 516:644], zsl(516, 644), AF.Tanh)              # wk
    nc.scalar.activation(gates[:, 773:901], zsl(773, 901), AF.Tanh)              # wv
    nc.scalar.activation(gates[:, 907:919], zsl(907, 919), AF.Identity)          # rm logits
    # softplus(z) = relu(z) + ln(1 + exp(-|z|)), grouped by activation function
    spts = {}
    for (a, b) in [(512, 516), (644, 645)]:
        spts[a] = scr.tile([BL, b - a], F32, tag="sp", name="spt")
        nc.scalar.activation(spts[a][:], zsl(a, b), AF.Abs)
    for (a, b) in [(512, 516), (644, 645)]:
        nc.scalar.activation(spts[a][:], spts[a][:], AF.Exp, scale=-1.0)
    for (a, b) in [(512, 516), (644, 645)]:
        nc.scalar.activation(spts[a][:], spts[a][:], AF.Ln, bias=1.0)
    for (a, b) in [(512, 516), (644, 645)]:
        nc.scalar.activation(gates[:, a:b], zsl(a, b), AF.Relu)
    for (a, b) in [(512, 516), (644, 645)]:
        nc.vector.tensor_tensor(gates[:, a:b], gates[:, a:b], spts[a][:], op=OP.add)

    # read-mode softmax over groups of 3
    rmz = gates[:, 907:919].rearrange("i (r k) -> i r k", k=3)
    negmax3 = P.tile([BL, R], F32, tag="negmax3")
    nc.vector.tensor_reduce(negmax3[:], rmz, axis=AX.X, op=OP.max, negate=True)
    rme = P.tile([BL, 3 * R], F32, tag="rme")
    nc.vector.tensor_tensor(rme[:].rearrange("i (r k) -> i r k", k=3), rmz,
                            negmax3[:].rearrange("i (r o) -> i r o", o=1).broadcast_to([BL, R, 3]),
                            op=OP.add)
    nc.scalar.activation(rme[:], rme[:], AF.Exp)
    rmsum = P.tile([BL, R], F32, tag="rmsum")
    nc.vector.tensor_reduce(rmsum[:], rme[:].rearrange("i (r k) -> i r k", k=3), axis=AX.X, op=OP.add)
    nc.vector.reciprocal(rmsum[:], rmsum[:])
    rm = P.tile([BL, 3 * R], F32, tag="rm")
    nc.vector.tensor_tensor(rm[:].rearrange("i (r k) -> i r k", k=3),
                            rme[:].rearrange("i (r k) -> i r k", k=3),
                            rmsum[:].rearrange("i (r o) -> i r o", o=1).broadcast_to([BL, R, 3]),
                            op=OP.mult)

    # per-item key-norm scalars (item-part)
    scw = P.tile([BL, W], F32, tag="scw")
    bw128 = P.tile([BL, 1], F32, tag="bw128")
    nc.scalar.activation(scw[:], gsl("wk"), AF.Square, accum_out=bw128[:])
    nc.scalar.activation(bw128[:], bw128[:], AF.Sqrt)
    nc.any.tensor_scalar(bw128[:], bw128[:], float(W), float(W) * DELTA, op0=OP.mult, op1=OP.add)
    bnr = P.tile([BL, R], F32, tag="bnr")
    rkwv = P.tile([BL, R], F32, tag="rkwv")
    for r in range(R):
        nc.scalar.activation(scw[:], gsl("rk", r * W, (r + 1) * W), AF.Square, accum_out=bnr[:, r:r + 1])
        nc.vector.tensor_tensor(scw[:], gsl("rk", r * W, (r + 1) * W), gsl("wv"), op=OP.mult)
        nc.vector.tensor_scalar(scw[:], scw[:], 1.0, None, op0=OP.mult, op1=OP.add,
                                accum_out=rkwv[:, r:r + 1])
    nc.scalar.activation(bnr[:], bnr[:], AF.Sqrt)
    nc.any.tensor_scalar(bnr[:], bnr[:], float(W), float(W) * DELTA, op0=OP.mult, op1=OP.add)
    c3 = P.tile([BL, 1], F32, tag="c3")
    nc.scalar.activation(scw[:], gsl("wv"), AF.Square, accum_out=c3[:])

    # KCM/NRM lhsT tables (w on partitions), f32 then cast to bf16
    KCMf = P.tile([128, BL * 11], F32, tag="KCMf")
    NRMf = P.tile([128, BL * 3], F32, tag="NRMf")
    EVT = P.tile([128, BL], F32, tag="EVT")

    def kcm_col(j):
        return KCMf[:].rearrange("p (i k) -> p i k", k=11)[:, :, j]

    gtp = pst(128, BL, psb)
    ptrans(gtp[:], gsl("wk"))
    nc.any.tensor_copy(kcm_col(0), gtp[:])
    gtp = pst(128, BL, psb)
    ptrans(gtp[:], gsl("ev"))
    nc.any.tensor_copy(EVT[:], gtp[:])
    gtp = pst(128, BL, psb)
    ptrans(gtp[:], gsl("wv"))
    nc.any.tensor_copy(kcm_col(9), gtp[:])
    nc.vector.tensor_tensor(kcm_col(10), kcm_col(9), EVT[:], op=OP.mult)  # ev*wv
    for r in range(R):
        gtp = pst(128, BL, psb)
        ptrans(gtp[:], gsl("rk", r * W, (r + 1) * W))
        nc.any.tensor_copy(kcm_col(1 + r), gtp[:])
        nc.vector.tensor_tensor(kcm_col(5 + r), kcm_col(1 + r), EVT[:], op=OP.mult)
    nrm3 = NRMf[:].rearrange("p (i k) -> p i k", k=3)
    nc.any.memset(nrm3[:, :, 0], 1.0)
    nc.any.tensor_copy(nrm3[:, :, 1], EVT[:])
    nc.scalar.activation(nrm3[:, :, 2], EVT[:], AF.Square)
    KCMb = P.tile([128, BL * 11], BF16, tag="KCMb")
    nc.any.tensor_copy(KCMb[:], KCMf[:])
    NRMb = P.tile([128, BL * 3], BF16, tag="NRMb")
    nc.any.tensor_copy(NRMb[:], NRMf[:])

    # ---------------- scalar table -> partition-broadcast SCB ----------------
    SCAL = P.tile([BL, NS], F32, tag="SCAL")
    nc.any.tensor_copy(SCAL[:, S_FG:S_FG + R], gsl("fg"))
    nc.any.tensor_copy(SCAL[:, S_RS:S_RS + R], gsl("rs"))
    nc.any.tensor_copy(SCAL[:, S_WS:S_WS + 1], gsl("ws"))
    nc.any.tensor_copy(SCAL[:, S_AG:S_AG + 1], gsl("ag"))
    nc.any.tensor_copy(SCAL[:, S_WG:S_WG + 1], gsl("wg"))
    for k in range(3):
        nc.any.tensor_copy(SCAL[:, S_M0 + R * k:S_M0 + R * (k + 1)],
                           rm[:].rearrange("i (r k) -> i r k", k=3)[:, :, k])
    nc.any.tensor_copy(SCAL[:, S_BNR:S_BNR + R], bnr[:])
    nc.any.tensor_copy(SCAL[:, S_BW:S_BW + 1], bw128[:])
    nc.any.tensor_copy(SCAL[:, S_RKWV:S_RKWV + R], rkwv[:])
    nc.any.tensor_copy(SCAL[:, S_C3:S_C3 + 1], c3[:])
    nc.any.tensor_scalar(SCAL[:, S_OMAG:S_OMAG + 1], gsl("ag"), -1.0, 1.0,
                         op0=OP.mult, op1=OP.add)
    scalrow = P.tile([1, BL * NS], F32, tag="scalrow")
    nc.scalar.dma_start(scalrow[:], SCAL[:])
    SCB = P.tile([128, BL * NS], F32, tag="SCB")
    nc.gpsimd.partition_broadcast(SCB[:], scalrow[:])

    def scb(s, w=1):
        # [128, BL, w] view of scalar cols s..s+w
        return SCB[:].rearrange("p (i s) -> p i s", s=NS)[:, :, s:s + w]

    def scb_c(s):
        # broadcast over chunks -> [128, NCH, BL]
        return SCB[:].rearrange("p (o i s) -> p o i s", o=1, s=NS)[:, :, :, s] \
            .broadcast_to([128, NCH, BL])

    def scb_cr(s):
        # per-(i,r) scalars broadcast over chunks -> [128, NCH, BL, R]
        return SCB[:].rearrange("p (o i s) -> p o i s", o=1, s=NS)[:, :, :, s:s + R] \
            .broadcast_to([128, NCH, BL, R])

    def bc_r(t):
        # [128, NCH*BL] -> [128, NCH, BL, R] broadcast over r
        return t[:].rearrange("p (c i o) -> p c i o", i=BL, o=1).broadcast_to([128, NCH, BL, R])

    if _bail(1):
        return

    # ---------------- phase A2: usage / u / log u (m-part) ----------------
    psi4 = sct(NCH * BL * R)
    nc.vector.tensor_tensor(vcir(psi4), vcir(RWT), scb_cr(S_FG), op=OP.mult)
    nc.any.tensor_scalar(psi4[:], psi4[:], 1.0, None, op0=OP.subtract)  # fg*rw - 1
    psi = sct()
    p4 = vcir(psi4)
    nc.vector.tensor_tensor(vci(psi), p4[:, :, :, 0], p4[:, :, :, 1], op=OP.mult)
    nc.vector.tensor_tensor(p4[:, :, :, 2], p4[:, :, :, 2], p4[:, :, :, 3], op=OP.mult)
    nc.vector.tensor_tensor(vci(psi), vci(psi), p4[:, :, :, 2], op=OP.mult)

    u_sb = P.tile([128, NCH * BL], F32, tag="u_sb")
    nc.vector.tensor_tensor(u_sb[:], USG[:], WWI[:], op=OP.mult)
    nc.vector.tensor_tensor(u_sb[:], USG[:], u_sb[:], op=OP.subtract)
    nc.vector.tensor_tensor(u_sb[:], u_sb[:], WWI[:], op=OP.add)
    nc.vector.tensor_tensor(u_sb[:], u_sb[:], psi[:], op=OP.mult)
    nc.any.tensor_scalar(u_sb[:], u_sb[:], 1.0 - DELTA, DELTA, op0=OP.mult, op1=OP.add)

    # u_ip (item-part copy of u) -> u_flat single row for mask broadcasts
    uips = pst(BL, M, psb)
    for c in range(NCH):
        ptrans(uips[:, 128 * c:128 * (c + 1)], vci(u_sb)[:, c, :])
    u_ip = P.tile([BL, M], F32, tag="u_ip")
    nc.any.tensor_copy(u_ip[:], uips[:])
    u_flat = P.tile([1, BL * M], F32, tag="u_flat")
    for j in range(8):
        nc.scalar.dma_start(u_flat[0:1, 1024 * j:1024 * (j + 1)], u_ip[2 * j:2 * (j + 1), :])

    if _bail(2, u_ip[:, 0:R * W]):
        return

    # ---------------- phase B + alloc mask (per item) ----------------
    BCOLL = P.tile([128, NCH * BL * 64], BF16, tag="BCOLL")
    S_acc = P.tile([128, NCH * BL], F32, tag="S_acc")
    stg2 = None
    for i in range(BL):
        if i % 2 == 0:
            mem2 = memp.tile([128, 2, M], BF16, tag="memt")
            nc.sync.dma_start(mem2[:], d["memt"][i:i + 2].rearrange("b w m -> w b m"))
            stg2 = stg.tile([128, M], BF16, tag="stg2", name="stgB")
            nc.gpsimd.memset(stg2[:], 0.0)
        mti = mem2[:, i % 2, :]
        mt2 = mtp.tile([128, M], BF16, tag="mt2")
        nc.gpsimd.tensor_tensor(mt2[:], mti, mti, op=OP.mult)
        kb_ps = pst(11, M)
        nc.tensor.matmul(kb_ps[:], KCMb[:, 11 * i:11 * (i + 1)], mti, start=True, stop=True)
        nr_ps = pst(3, M)
        nc.tensor.matmul(nr_ps[:], NRMb[:, 3 * i:3 * (i + 1)], mt2[:], start=True, stop=True)
        o = 64 * (i % 2)
        nc.any.tensor_copy(stg2[o:o + 11, :], kb_ps[:])
        nc.any.tensor_copy(stg2[o + 32:o + 35, :], nr_ps[:])
        if i % 2 == 1:
            dst = BCOLL[:].rearrange("p (c i k) -> p c (i k)", i=BL, k=64)[:, :, 64 * (i - 1):64 * (i + 1)]
            nc.sync.dma_start_transpose(dst, stg2[:])
        # --- mask for item i ---
        urep = urp.tile([128, M], F32, tag="urep")
        nc.gpsimd.partition_broadcast(urep[:], u_flat[0:1, M * i:M * (i + 1)])
        lurep = urp.tile([128, M], F32, tag="lurep")
        nc.scalar.activation(lurep[:], urep[:], AF.Ln)
        mscr = scr.tile([128, M], F32, tag="mscr", name="mscr")
        for c in range(NCH):
            nc.vector.scalar_tensor_tensor(mscr[:], urep[:], vci(u_sb)[:, c, i:i + 1],
                                           lurep[:], op0=OP.is_lt, op1=OP.mult,
                                           accum_out=vci(S_acc)[:, c, i:i + 1])

    if _bail(3, u_ip[:, 0:R * W]):
        return

    # ---------------- phase C: batched m-part weights ----------------
    bcf = vcir(BCOLL, 64)

    def bcol(j):
        return bcf[:, :, :, j]

    WCN, T1, T2, S0, S1, S2 = bcol(0), bcol(9), bcol(10), bcol(32), bcol(33), bcol(34)

    # alloc = (1-u) * exp(S)
    EXS = sct()
    nc.scalar.activation(EXS[:], S_acc[:], AF.Exp)
    onemu = sct()
    nc.any.tensor_scalar(onemu[:], u_sb[:], -1.0, 1.0, op0=OP.mult, op1=OP.add)
    alloc = sct()
    nc.vector.tensor_tensor(alloc[:], onemu[:], EXS[:], op=OP.mult)

    # write content weights wcw (no max subtraction; logits are tiny)
    AO = sct()
    nc.scalar.activation(vci(AO), S0, AF.Sqrt)
    nc.any.tensor_scalar(AO[:], AO[:], 1.0, DELTA, op0=OP.mult, op1=OP.add)
    wden = sct()
    nc.vector.tensor_tensor(vci(wden), vci(AO), scb_c(S_BW), op=OP.mult)
    nc.any.tensor_scalar(wden[:], wden[:], 1.0, DELTA, op0=OP.mult, op1=OP.add)
    nc.vector.reciprocal(wden[:], wden[:])
    wz = sct()
    nc.vector.tensor_tensor(vci(wz), WCN, vci(wden), op=OP.mult)
    nc.vector.tensor_tensor(vci(wz), vci(wz), scb_c(S_WS), op=OP.mult)
    nc.scalar.activation(wz[:], wz[:], AF.Exp)
    wzs = P.tile([128, BL], F32, tag="wzs")
    nc.vector.tensor_reduce(wzs[:], wz[:].rearrange("p (c i) -> p i c", i=BL), axis=AX.X, op=OP.add)
    WZS = P.tile([128, BL], F32, tag="WZS")
    nc.gpsimd.partition_all_reduce(WZS[:], wzs[:], channels=128, reduce_op=bass_isa.ReduceOp.add)
    nc.vector.reciprocal(WZS[:], WZS[:])
    wcw = sct()
    nc.vector.tensor_tensor(vci(wcw), vci(wz),
                            WZS[:].rearrange("p (o i) -> p o i", o=1).broadcast_to([128, NCH, BL]),
                            op=OP.mult)

    # write weights ww = wg * (ag*alloc + (1-ag)*wcw)
    ww = P.tile([128, NCH * BL], F32, tag="ww")
    nc.vector.tensor_tensor(vci(alloc), vci(alloc), scb_c(S_AG), op=OP.mult)
    nc.vector.tensor_tensor(vci(ww), vci(wcw), scb_c(S_OMAG), op=OP.mult)
    nc.vector.tensor_tensor(ww[:], ww[:], alloc[:], op=OP.add)
    nc.vector.tensor_tensor(vci(ww), vci(ww), scb_c(S_WG), op=OP.mult)

    if _bail(4, u_ip[:, 0:R * W]):
        return

    # fp8 lhsT combos for link matmuls: [rw*512 | rw*512*ww]
    RWC8 = P.tile([128, NCH * BL * 8], FP8, tag="RWC8")
    rwc = vcir(RWC8, 8)
    nc.any.tensor_scalar(rwc[:, :, :, 0:4], vcir(RWT), LSC, None, op0=OP.mult)
    nc.vector.scalar_tensor_tensor(rwc[:, :, :, 4:8], vcir(RWT), LSC, bc_r(ww),
                                   op0=OP.mult, op1=OP.mult)

    if _bail(5, u_ip[:, 0:R * W]):
        return

    # ---------------- phase D: link products (per item) ----------------
    DCOLL = P.tile([128, NCH * BL * 64], BF16, tag="DCOLL")
    stg2d = None
    for i in range(BL):
        ll = lp.tile([128, 2, NCH, M], FP8, tag="ll")
        nc.sync.dma_start(ll[:], d["llt"][i].rearrange("g (c p) n -> p g c n", p=128))
        if i % 2 == 0:
            stg2d = stg.tile([128, M], BF16, tag="stg2", name="stgD")
            nc.gpsimd.memset(stg2d[:], 0.0)
        bps = pst(8, M)
        fps = pst(8, M)
        for c in range(NCH):
            lhs = RWC8[:].rearrange("p (c i k) -> p c i k", i=BL, k=8)[:, c, i, :]
            nc.tensor.matmul(bps[:], lhs, ll[:, 0, c, :], start=(c == 0), stop=(c == NCH - 1))
            nc.tensor.matmul(fps[:], lhs, ll[:, 1, c, :], start=(c == 0), stop=(c == NCH - 1))
        o = 64 * (i % 2)
        nc.any.tensor_scalar(stg2d[o:o + 8, :], bps[:], LDS, None, op0=OP.mult)
        nc.any.tensor_scalar(stg2d[o + 32:o + 40, :], fps[:], LDS, None, op0=OP.mult)
        if i % 2 == 1:
            dst = DCOLL[:].rearrange("p (c i k) -> p c (i k)", i=BL, k=64)[:, :, 64 * (i - 1):64 * (i + 1)]
            nc.sync.dma_start_transpose(dst, stg2d[:])

    if _bail(6, u_ip[:, 0:R * W]):
        return

    # new-memory norms AN
    ww2 = sct()
    nc.scalar.activation(ww2[:], ww[:], AF.Square)
    q1 = sct()
    nc.vector.tensor_tensor(vci(q1), S1, T1, op=OP.subtract)
    nc.vector.tensor_tensor(q1[:], ww[:], q1[:], op=OP.mult)
    q2 = sct()
    nc.any.tensor_scalar(vci(q2), T2, -2.0, None, op0=OP.mult)
    nc.vector.tensor_tensor(vci(q2), vci(q2), S2, op=OP.add)
    nc.vector.tensor_tensor(vci(q2), vci(q2), scb_c(S_C3), op=OP.add)
    nc.vector.tensor_tensor(q2[:], q2[:], ww2[:], op=OP.mult)
    AN = sct()
    nc.any.tensor_scalar(q1[:], q1[:], -2.0, None, op0=OP.mult)
    nc.vector.tensor_tensor(vci(AN), S0, vci(q1), op=OP.add)
    nc.vector.tensor_tensor(AN[:], AN[:], q2[:], op=OP.add)
    nc.scalar.activation(AN[:], AN[:], AF.Sqrt)
    nc.any.tensor_scalar(AN[:], AN[:], 1.0, DELTA, op0=OP.mult, op1=OP.add)

    # read content weights cw (scaled by mode2 / csum)
    cnum = sct(NCH * BL * R)
    cn = vcir(cnum)
    cwA = bcf[:, :, :, 1:5]
    cwB = bcf[:, :, :, 5:9]  # bf16 views read directly
    nc.vector.tensor_tensor(cn, cwB, bc_r(ww), op=OP.mult)
    nc.vector.tensor_tensor(cn, cwA, cn, op=OP.subtract)
    ct = sct(NCH * BL * R)
    nc.vector.tensor_tensor(vcir(ct), bc_r(ww), scb_cr(S_RKWV), op=OP.mult)
    nc.vector.tensor_tensor(cnum[:], cnum[:], ct[:], op=OP.add)
    cden = sct(NCH * BL * R)
    nc.vector.tensor_tensor(vcir(cden), bc_r(AN), scb_cr(S_BNR), op=OP.mult)
    nc.any.tensor_scalar(cden[:], cden[:], 1.0, DELTA, op0=OP.mult, op1=OP.add)
    nc.vector.reciprocal(cden[:], cden[:])
    nc.vector.tensor_tensor(cnum[:], cnum[:], cden[:], op=OP.mult)
    nc.vector.tensor_tensor(cn, cn, scb_cr(S_RS), op=OP.mult)
    nc.scalar.activation(cnum[:], cnum[:], AF.Exp)
    csum = P.tile([128, BL * R], F32, tag="csum")
    nc.vector.tensor_reduce(csum[:], cnum[:].rearrange("p (c j) -> p j c", j=BL * R),
                            axis=AX.X, op=OP.add)
    CSR = P.tile([128, BL * R], F32, tag="CSR")
    nc.gpsimd.partition_all_reduce(CSR[:], csum[:], channels=128, reduce_op=bass_isa.ReduceOp.add)
    nc.vector.reciprocal(CSR[:], CSR[:])
    nc.vector.tensor_tensor(CSR[:].rearrange("p (i r) -> p i r", r=R),
                            CSR[:].rearrange("p (i r) -> p i r", r=R),
                            scb(S_M2, R), op=OP.mult)
    cwm2 = sct(NCH * BL * R)   # mode2 * cw
    nc.vector.tensor_tensor(vcir(cwm2), cn,
                            CSR[:].rearrange("p (o i r) -> p o i r", o=1, r=R)
                            .broadcast_to([128, NCH, BL, R]), op=OP.mult)


    # ---------------- phase E: assemble fwd/bwd/rw_new (m-part) ----------------
    dcv = vcir(DCOLL, 64)
    P1, P2 = dcv[:, :, :, 0:4], dcv[:, :, :, 4:8]
    F1, F2 = dcv[:, :, :, 32:36], dcv[:, :, :, 36:40]

    # cpr = prec . rw_r ; dwr = rw_r . ww   (per item, read head)
    scr4 = sct(NCH * BL * R)
    prcb = PRC[:].rearrange("p (c i o) -> p c i o", i=BL, o=1).broadcast_to([128, NCH, BL, R])
    nc.vector.tensor_tensor(vcir(scr4), vcir(RWT), prcb, op=OP.mult)
    CDW = P.tile([128, 2 * BL * R], F32, tag="CDW")
    nc.vector.tensor_reduce(CDW[:, 0:BL * R], scr4[:].rearrange("p (c j) -> p j c", j=BL * R),
                            axis=AX.X, op=OP.add)
    scr4b = sct(NCH * BL * R)
    nc.vector.tensor_tensor(vcir(scr4b), vcir(RWT), bc_r(ww), op=OP.mult)
    nc.vector.tensor_reduce(CDW[:, BL * R:2 * BL * R],
                            scr4b[:].rearrange("p (c j) -> p j c", j=BL * R),
                            axis=AX.X, op=OP.add)
    CDWr = P.tile([128, 2 * BL * R], F32, tag="CDWr")
    nc.gpsimd.partition_all_reduce(CDWr[:], CDW[:], channels=128, reduce_op=bass_isa.ReduceOp.add)

    def cdw_b(off):
        return CDWr[:, off:off + BL * R].rearrange("p (o i r) -> p o i r", o=1, r=R) \
            .broadcast_to([128, NCH, BL, R])

    # dv = (1-2ww)*diag + ww*prec ; DR = rw * dv
    dv = sct()
    nc.any.tensor_scalar(dv[:], ww[:], -2.0, 1.0, op0=OP.mult, op1=OP.add)
    nc.vector.tensor_tensor(dv[:], dv[:], DGT[:], op=OP.mult)
    t2m = sct()
    nc.vector.tensor_tensor(t2m[:], ww[:], PRC[:], op=OP.mult)
    nc.vector.tensor_tensor(dv[:], dv[:], t2m[:], op=OP.add)
    DR = sct(NCH * BL * R)
    nc.vector.tensor_tensor(vcir(DR), vcir(RWT), bc_r(dv), op=OP.mult)

    onemw = sct()
    nc.any.tensor_scalar(onemw[:], ww[:], -1.0, 1.0, op0=OP.mult, op1=OP.add)

    # fwd = F1*(1-ww) - F2 + ww (x) cpr - DR   (then scaled by mode1)
    fwd = sct(NCH * BL * R)
    fv = vcir(fwd)
    nc.vector.tensor_tensor(fv, F1, bc_r(onemw), op=OP.mult)
    nc.vector.tensor_tensor(fv, fv, F2, op=OP.subtract)
    ftt = sct(NCH * BL * R)
    nc.vector.tensor_tensor(vcir(ftt), bc_r(ww), cdw_b(0), op=OP.mult)
    nc.vector.tensor_tensor(fwd[:], fwd[:], ftt[:], op=OP.add)
    nc.vector.tensor_tensor(fwd[:], fwd[:], DR[:], op=OP.subtract)

    # bwd = P1*(1-ww) - P2 + prec (x) dwr - DR  (then scaled by mode0)
    bwd = sct(NCH * BL * R)
    bv = vcir(bwd)
    nc.vector.tensor_tensor(bv, P1, bc_r(onemw), op=OP.mult)
    nc.vector.tensor_tensor(bv, bv, P2, op=OP.subtract)
    nc.vector.tensor_tensor(vcir(ftt), prcb, cdw_b(BL * R), op=OP.mult)
    nc.vector.tensor_tensor(bwd[:], bwd[:], ftt[:], op=OP.add)
    nc.vector.tensor_tensor(bwd[:], bwd[:], DR[:], op=OP.subtract)

    rwnew = P.tile([128, NCH * BL * R], F32, tag="rwnew")
    nc.vector.tensor_tensor(bv, bv, scb_cr(S_M0), op=OP.mult)
    nc.vector.tensor_tensor(fv, fv, scb_cr(S_M1), op=OP.mult)
    nc.vector.tensor_tensor(rwnew[:], bwd[:], fwd[:], op=OP.add)
    nc.vector.tensor_tensor(rwnew[:], rwnew[:], cwm2[:], op=OP.add)

    # sc = rwnew . ww
    nc.vector.tensor_tensor(vcir(scr4), vcir(rwnew), bc_r(ww), op=OP.mult)
    SC1 = P.tile([128, BL * R], F32, tag="SC1")
    nc.vector.tensor_reduce(SC1[:], scr4[:].rearrange("p (c j) -> p j c", j=BL * R),
                            axis=AX.X, op=OP.add)
    SCR_ = P.tile([128, BL * R], F32, tag="SCR_")
    nc.gpsimd.partition_all_reduce(SCR_[:], SC1[:], channels=128, reduce_op=bass_isa.ReduceOp.add)

    # bf16 lhsT for read vectors: [rwnew | rwnew*ww]
    RVL = P.tile([128, NCH * BL * 8], BF16, tag="RVL")
    rvv = vcir(RVL, 8)
    nc.any.tensor_copy(rvv[:, :, :, 0:4], vcir(rwnew))
    nc.vector.tensor_tensor(rvv[:, :, :, 4:8], vcir(rwnew), bc_r(ww), op=OP.mult)

    if _bail(7, u_ip[:, 0:R * W]):
        return

    # ---------------- phase F: read vectors (per item) ----------------
    rv16 = P.tile([BL, 2 * R * W], F32, tag="rv16")
    for i in range(BL):
        if i % 2 == 0:
            mn2 = memp.tile([128, 2, NCH, W], BF16, tag="memn")
            nc.sync.dma_start(mn2[:], d["memn"][i:i + 2].rearrange("b (c p) w -> p b c w", p=128))
        rvp = pst(8, W)
        for c in range(NCH):
            nc.tensor.matmul(rvp[:], rvv[:, c, i, :], mn2[:, i % 2, c, :],
                             start=(c == 0), stop=(c == NCH - 1))
        rvs = stg.tile([8, W], F32, tag="rvs", name="rvs")
        nc.any.tensor_copy(rvs[:], rvp[:])
        nc.scalar.dma_start(rv16[i:i + 1, :], rvs[:])

    if _bail(8, rv16[:, 0:R * W]):
        return

    # ---------------- phase G: final combine (item-part) ----------------
    sc_ip = P.tile([BL, R], F32, tag="sc_ip")
    nc.sync.dma_start(sc_ip[:], SCR_[0:1, :])

    def g4(t, n=W):
        return t.rearrange("i (r m) -> i r m", m=n)

    def bcm_w(t):
        return t.rearrange("i (o m) -> i o m", o=1).broadcast_to([BL, R, W])

    def bc4_w(t):
        return t.rearrange("i (r o) -> i r o", o=1).broadcast_to([BL, R, W])

    RA = g4(rv16[:, 0:R * W])
    RB = g4(rv16[:, R * W:2 * R * W])
    out_sb = P.tile([BL, R * W], F32, tag="out_sb")
    nc.vector.tensor_tensor(g4(out_sb[:]), RB, bcm_w(gsl("ev")), op=OP.mult)
    nc.vector.tensor_tensor(g4(out_sb[:]), RA, g4(out_sb[:]), op=OP.subtract)
    wvt = P.tile([BL, R * W], F32, tag="wvt")
    nc.vector.tensor_tensor(g4(wvt[:]), bcm_w(gsl("wv")), bc4_w(sc_ip[:]), op=OP.mult)
    nc.vector.tensor_tensor(out_sb[:], out_sb[:], wvt[:], op=OP.add)
    nc.sync.dma_start(d["out"][:], out_sb[:])


_NC_CACHE = {}


def build_nc():
    if "nc" in _NC_CACHE:
        return _NC_CACHE["nc"]
    nc = bacc.Bacc("TRN2", target_bir_lowering=False, debug=False)
    d = {}
    d["consts"] = nc.dram_tensor("consts", [128, 129], F32, kind="ExternalInput")
    d["xta"] = nc.dram_tensor("xta", [KIN, BL], BF16, kind="ExternalInput")
    d["wta"] = nc.dram_tensor("wta", [KIN, DTOT], BF16, kind="ExternalInput")
    d["memt"] = nc.dram_tensor("memt", [BL, W, M], BF16, kind="ExternalInput")
    d["memn"] = nc.dram_tensor("memn", [BL, M, W], BF16, kind="ExternalInput")
    d["llt"] = nc.dram_tensor("llt", [BL, 2, M, M], FP8, kind="ExternalInput")
    d["rwt"] = nc.dram_tensor("rwt", [M, BL * R], F32, kind="ExternalInput")
    d["prct"] = nc.dram_tensor("prct", [M, BL], F32, kind="ExternalInput")
    d["wwt"] = nc.dram_tensor("wwt", [M, BL], F32, kind="ExternalInput")
    d["usgt"] = nc.dram_tensor("usgt", [M, BL], F32, kind="ExternalInput")
    d["diagt"] = nc.dram_tensor("diagt", [M, BL], F32, kind="ExternalInput")
    d["out"] = nc.dram_tensor("out", [BL, R * W], F32, kind="ExternalOutput")
    with tile.TileContext(nc) as tc:
        with ExitStack() as ctx:
            _emit(nc, tc, ctx, d)
    nc.compile()
    _NC_CACHE["nc"] = nc
    return nc


def make_in_maps(inputs):
    names = ["rk", "rs", "wk", "ws", "ev", "wv", "fg", "ag", "wg", "rm"]
    Wall = np.concatenate([np.asarray(inputs[f"W_{n}"]) for n in names], axis=0).astype(np.float32)
    ball = np.concatenate([np.asarray(inputs[f"b_{n}"]) for n in names], axis=0).astype(np.float32)
    wta = np.zeros((KIN, DTOT), np.float32)
    wta[:IN] = Wall.T
    wta[IN] = ball
    wta = wta.astype(ml_dtypes.bfloat16)
    consts = np.zeros((128, 129), np.float32)
    consts[:, :128] = np.eye(128, dtype=np.float32)
    consts[:, 128] = 1.0

    x = np.asarray(inputs["x"], np.float32)
    mem = np.asarray(inputs["memory"], np.float32)
    link = np.asarray(inputs["link_matrix"], np.float32)[:, 0]
    prec = np.asarray(inputs["precedence"], np.float32)[:, 0]
    rw = np.asarray(inputs["read_weights"], np.float32)
    wwin = np.asarray(inputs["write_weights"], np.float32)[:, 0]
    usage = np.asarray(inputs["usage_vector"], np.float32)

    in_maps = []
    for cix in range(NCORES):
        sl = slice(cix * BL, (cix + 1) * BL)
        xta = np.zeros((KIN, BL), np.float32)
        xta[:IN] = x[sl].T
        xta[IN] = 1.0
        rws = rw[sl]
        lk = link[sl]
        llt = np.stack([lk, lk.transpose(0, 2, 1)], axis=1) * LSC
        diag = np.ascontiguousarray(np.diagonal(lk, axis1=1, axis2=2))
        in_maps.append({
            "consts": consts,
            "xta": xta.astype(ml_dtypes.bfloat16),
            "wta": wta,
            "memt": np.ascontiguousarray(mem[sl].transpose(0, 2, 1)).astype(ml_dtypes.bfloat16),
            "memn": np.ascontiguousarray(mem[sl]).astype(ml_dtypes.bfloat16),
            "llt": np.ascontiguousarray(llt).astype(ml_dtypes.float8_e4m3fn),
            "rwt": np.ascontiguousarray(rws.transpose(2, 0, 1).reshape(M, BL * R)),
            "prct": np.ascontiguousarray(prec[sl].T),
            "wwt": np.ascontiguousarray(wwin[sl].T),
            "usgt": np.ascontiguousarray(usage[sl].T),
            "diagt": np.ascontiguousarray(diag.T),
        })
    return in_maps


def kernel(**inputs):
    nc = build_nc()
    in_maps = make_in_maps(inputs)
    res = run_bass_kernel_spmd(nc, in_maps, list(range(NCORES))).results
    out = np.concatenate([res[c]["out"].reshape(BL, R, W) for c in range(NCORES)], axis=0)
    return out.astype(np.float32)
